# revision 2
# baseline (speedup 1.0000x reference)
"""Bidirectional spatial Mamba block on 8 Trainium2 NeuronCores — v2.

Sharding: core c = b*4 + dir*2 + half handles batch b, scan direction dir
(backward cores get host-reversed input; host un-reverses their output),
and d-half `half` of the DIN=192 inner channels. Each core runs an identical
SPMD program producing a [96, L] partial of out_w @ y_dir; the host sums the
four partials per batch and adds the residual x.

v2 design (vs baseline):
- (d,n) packed scan: the 96 d-channels x 16 states = 1536 recurrence rows are
  packed into 12 tiles of 128 partitions (p = 16*q + n, d = 8k + q), so each
  chunk runs 12 tensor_tensor_scans of [128, T] instead of 16 of [96, T].
- bf16 everywhere except the scan decay path (delta kept fp32-relative).
- All matmuls bf16 (fp32 matmul is 2 instructions and ~3x slower).
- GpSimd left idle: Pool ops contend with DVE for the SBUF port and slow
  the scans down ~40%.
- LN affine, conv bias, and the -mu*rstd LN term are folded into the fused
  in_proj+conv matmul via two extra input rows (ones row, mrs row).
- B/C/dl broadcasts into the packed layout via PE selector matmuls;
  dx replication via a DRAM round-trip DMA (write-side 16x replication).
- y = sum_n C_n*h_n accumulated in PSUM by 12 selector matmuls + a diag(D)
  matmul (the D*xc term), so no elementwise adds are needed.
"""
import numpy as np

import concourse.bass as bass
import concourse.mybir as mybir
import concourse.tile as tile
from concourse.bass_utils import run_bass_kernel_spmd

AF = mybir.ActivationFunctionType
OP = mybir.AluOpType
FP32 = mybir.dt.float32
BF16 = mybir.dt.bfloat16

CH, DIN, NST, DTR, DCONV = 96, 192, 16, 6, 4
B, H, W = 2, 128, 128
L = H * W
T = 512
NK = 12          # (d,n) tiles of 128 partitions: 96*16 / 128
NBC = DTR + 2 * NST   # 38 x_proj rows


# ---------------------------------------------------------------- tile patch
# This walrus codegen rejects the multi-wait Drain that TileContext emits at
# exit ("Too many sync wait commands"); split the waits onto single-wait NoOps.
_PATCHED = False


def _patch_tile_drain():
    global _PATCHED
    if _PATCHED:
        return
    _PATCHED = True
    from bass_rust import ScopedClock

    def patched(self, tick_clock, wait_clock):
        nc = self.nc
        carrier = nc.sync.nop()
        wait_clock.add_sem_waits(
            carrier.ins, ScopedClock({None: tick_clock.global_clock})
        )
        si = carrier.ins.sync_info
        waits = list(si.on_wait) if si is not None else []
        if si is not None:
            si.on_wait = waits[:1]
            for w in waits[1:]:
                n2 = nc.sync.nop()
                n2.ins.sync_info = mybir.SyncInfo(on_wait=[w], on_update=[])
        nc.sync.drain()
        nc.all_engine_barrier()
        assert self.sems is not None
        popped = nc._tile_sem_poison_stack.pop()
        assert popped is self._sem_poison
        nc.clear_and_free_semaphores(list(self.sems.allocated().values()))
        nc.all_engine_barrier()

    tile.TileContext._drain_and_barrier = patched


def _split_waits(nc, max_waits=1):
    """Walrus rejects instructions carrying more than ~1 sem wait. Hoist
    extras onto same-engine NoOp carriers inserted just before."""
    for bb in nc.main_func.blocks:
        new_insts = []
        for ins in bb.instructions:
            si = ins.sync_info
            if si is not None and len(si.on_wait) > max_waits:
                waits = list(si.on_wait)
                for w in waits[max_waits:]:
                    nop = mybir.InstNoOp(
                        name=nc.get_next_instruction_name(),
                        engine=ins.engine, ins=[], outs=[],
                        sync_info=mybir.SyncInfo(on_wait=[w], on_update=[]),
                    )
                    nc.register_instruction(nop)
                    new_insts.append(nop)
                si.on_wait = waits[:max_waits]
            new_insts.append(ins)
        bb.instructions[:] = new_insts


# ---------------------------------------------------------------- builder
def build_program(nchunk=L // T):
    _patch_tile_drain()
    nc = bass.Bass(num_devices=8)
    nc.allow_non_contiguous_dma("broadcast/replication DMAs")
    # const AP so activation(bias=const) lowers
    eps_t = nc.alloc_sbuf_tensor("const-f32-lneps", [128, 1], FP32)
    nc.gpsimd.memset(eps_t.ap(), 1e-5)
    nc.const_aps.aps[(FP32, 1e-5)] = eps_t.ap()
    one_t = nc.alloc_sbuf_tensor("const-f32-one", [128, 1], FP32)
    nc.gpsimd.memset(one_t.ap(), 1.0)
    nc.const_aps.aps[(FP32, 1.0)] = one_t.ap()
    nc.all_engine_barrier()
    Ltot = nchunk * T

    din = {}
    for name, shape, dt in [
        ("xin", [CH, Ltot], BF16),
        ("wcv", [98, 8 * 96], BF16),      # fused in_proj+conv lhsT, 4 taps x 2 halves
        ("wz", [98, 96], BF16),
        ("wxp", [96, 2 * NBC], BF16),     # x_proj lhsT, K-split halves
        ("wdt", [DTR, 96], BF16),
        ("dtb", [CH, 1], FP32),
        ("seldl", [96, NK * 128], BF16),  # d -> packed(p) replication
        ("selbc", [NBC, 2 * 128], BF16),  # B,C row -> packed(p)
        ("ascv", [128, NK], FP32),        # A[d(p,k), n(p)]
        ("dvm", [96, 96], BF16),          # diag(D)
        ("sely", [128, NK * 96], BF16),   # packed(p) -> d contraction
        ("wout", [96, 96], BF16),
        ("ones1", [CH, 1], BF16),
        ("ones_r", [1, CH], BF16),
    ]:
        din[name] = nc.declare_dram_parameter(name, shape, dt, isOutput=False)
    pout = nc.declare_dram_parameter("pout", [CH, Ltot], BF16, isOutput=True)
    # dram scratch for dx replication round-trip (write-side 16x replication)
    dxscr = nc.declare_dram_parameter("dxscr", [NK * 128, T], BF16, isOutput=True)

    with tile.TileContext(nc) as tc:
        with (
            tc.tile_pool(name="const", bufs=1) as const,
            tc.tile_pool(name="io", bufs=3) as io,
            tc.tile_pool(name="work", bufs=2) as work,
            tc.tile_pool(name="small", bufs=2) as small,
            tc.tile_pool(name="big", bufs=2) as big,
            tc.tile_pool(name="ps_a", bufs=2, space="PSUM") as ps_a,
            tc.tile_pool(name="ps_mm", bufs=2, space="PSUM") as ps_mm,
            tc.tile_pool(name="ps_proj", bufs=2, space="PSUM") as ps_proj,
            tc.tile_pool(name="ps_po", bufs=1, space="PSUM") as ps_po,
            tc.tile_pool(name="ps_y", bufs=1, space="PSUM") as ps_y,
        ):
            cst = {}
            for name in ["wcv", "wz", "wxp", "wdt", "dtb", "seldl", "selbc",
                         "ascv", "dvm", "sely", "wout", "ones1", "ones_r"]:
                t = const.tile(list(din[name].shape), din[name].dtype,
                               tag=name, name=name)
                nc.gpsimd.dma_start(t[:], din[name].ap()[:])
                cst[name] = t

            # persistent ping-pong xn tiles [98, T+3]: rows 0:96 = xhat,
            # row 96 = mu*sum(gam*W) fold (mrs), row 97 = ones (bias fold)
            xn_tiles = [const.tile([98, T + 3], BF16, tag=f"xn{i}",
                                   name=f"xn{i}") for i in range(2)]
            for t_ in xn_tiles:
                nc.vector.memset(t_[:], 0.0)
                nc.vector.memset(t_[96:98, 3:T + 3], 1.0)
            # hn ping-pong [128, NK*T]
            hn_tiles = [const.tile([128, NK * T], BF16, tag=f"hn{i}",
                                   name=f"hn{i}") for i in range(2)]
            nc.vector.memset(hn_tiles[1][:], 0.0)

            # cross-stage state; emission order IS dependency order in Tile,
            # so every consumer is emitted after its producer. Front ops of
            # chunk ci are woven between the scans of chunk ci-2 such that
            # each engine stream rarely waits.
            stash = {}

            def seg_a(ci):
                """LN head: input DMA + sums (Act/PE)."""
                sl = bass.ts(ci, T)
                xt = io.tile([CH, T], BF16, tag="xt")
                nc.gpsimd.dma_start(xt[:], din["xin"].ap()[:, sl])
                xsq = work.tile([CH, T], BF16, tag="xsq")
                nc.scalar.square(xsq[:], xt[:])
                s1 = ps_a.tile([1, T], FP32, tag="psa")
                nc.tensor.matmul(s1[:], cst["ones1"][:], xt[:])
                musq = small.tile([1, T], FP32, tag="musq")
                nc.scalar.activation(musq[:], s1[:], AF.Square, scale=1.0 / CH)
                s2 = ps_a.tile([1, T], FP32, tag="psa")
                nc.tensor.matmul(s2[:], cst["ones1"][:], xsq[:])
                stash[("f", ci)] = f = {}
                f.update(xt=xt, s1=s1, s2=s2, musq=musq)

            def seg_var(ci):
                f = stash[("f", ci)]
                var = small.tile([1, T], FP32, tag="var")
                nc.vector.scalar_tensor_tensor(
                    var[:], f["s2"][:], 1.0 / CH, f["musq"][:],
                    OP.mult, OP.subtract)
                f["var"] = var

            def seg_rstd(ci):
                f = stash[("f", ci)]
                lv = small.tile([1, T], FP32, tag="lv")
                nc.scalar.activation(lv[:], f["var"][:], AF.Ln, bias=1e-5)
                rstd = small.tile([1, T], BF16, tag="rstd")
                nc.scalar.activation(rstd[:], lv[:], AF.Exp, scale=-0.5)
                rstd_b = ps_a.tile([96, T], FP32, tag="psa")
                nc.tensor.matmul(rstd_b[:], cst["ones_r"][:], rstd[:])
                f.update(rstd=rstd, rstd_b=rstd_b)

            def seg_xn(ci):
                """mrs row, xhat rows, halo (DVE)."""
                f = stash[("f", ci)]
                xn = xn_tiles[ci % 2]
                xn_nxt = xn_tiles[(ci + 1) % 2]
                nc.vector.tensor_mul(xn[96:97, 3:T + 3], f["s1"][:], f["rstd"][:])
                nc.vector.tensor_mul(xn[0:96, 3:T + 3], f["xt"][:], f["rstd_b"][:])
                nc.vector.tensor_copy(xn_nxt[:, 0:3], xn[:, T:T + 3])

            def seg_conv(ci):
                """conv/z matmuls + the three sigmoid chains (PE/Act)."""
                f = stash[("f", ci)]
                xn = xn_tiles[ci % 2]
                cps = []
                for hf in range(2):
                    cp = ps_mm.tile([96, T], FP32, tag="mm")
                    for j in range(4):
                        nc.tensor.matmul(
                            cp[:],
                            cst["wcv"][:, (4 * hf + j) * 96:(4 * hf + j + 1) * 96],
                            xn[:, j:j + T],
                            start=(j == 0), stop=(j == 3))
                    cps.append(cp)
                zps = ps_mm.tile([96, T], FP32, tag="mm")
                nc.tensor.matmul(zps[:], cst["wz"][:], xn[:, 3:T + 3])
                sgs = []
                for idx, ps in enumerate(cps + [zps]):
                    en1 = work.tile([96, T], FP32, tag="en1",
                                    name=f"en1_{idx}", bufs=2)
                    nc.scalar.activation(en1[:], ps[:], AF.Exp, scale=-1.0)
                    nc.scalar.activation(en1[:], en1[:], AF.Ln, bias=1.0)
                    sg = work.tile([96, T], BF16, tag="sg",
                                   name=f"sg_{idx}", bufs=3)
                    nc.scalar.activation(sg[:], en1[:], AF.Exp, scale=-1.0)
                    sgs.append(sg)
                f.update(cps=cps, zps=zps, sgs=sgs)

            def seg_silu_mul(ci):
                f = stash[("f", ci)]
                xcA = work.tile([96, T], BF16, tag="xcA", bufs=3)
                xcB = work.tile([96, T], BF16, tag="xcB", bufs=2)
                zs = work.tile([96, T], BF16, tag="zs", bufs=3)
                nc.vector.tensor_mul(xcA[:], f["cps"][0][:], f["sgs"][0][:])
                nc.vector.tensor_mul(xcB[:], f["cps"][1][:], f["sgs"][1][:])
                nc.vector.tensor_mul(zs[:], f["zps"][:], f["sgs"][2][:])
                f.update(xcA=xcA, xcB=xcB, zs=zs)

            def seg_proj(ci):
                """x_proj + delta chain (PE/Act)."""
                f = stash[("f", ci)]
                dbl = ps_proj.tile([NBC, T], FP32, tag="proj")
                nc.tensor.matmul(dbl[:], cst["wxp"][:, 0:NBC], f["xcA"][:],
                                 start=True, stop=False)
                nc.tensor.matmul(dbl[:], cst["wxp"][:, NBC:2 * NBC], f["xcB"][:],
                                 start=False, stop=True)
                dblb = work.tile([NBC, T], BF16, tag="dblb")
                nc.scalar.copy(dblb[:], dbl[:])
                dpre = ps_proj.tile([96, T], FP32, tag="proj")
                nc.tensor.matmul(dpre[:], cst["wdt"][:], dblb[0:DTR, :])
                spe = work.tile([96, T], FP32, tag="spe")
                nc.scalar.activation(spe[:], dpre[:], AF.Exp, bias=cst["dtb"][:])
                dlb = work.tile([96, T], BF16, tag="dlb")
                nc.scalar.activation(dlb[:], spe[:], AF.Ln, bias=1.0)
                f.update(dblb=dblb, dlb=dlb)

            def seg_dx(ci):
                """dx mul (DVE), replication DMAs, B/C selector matmuls."""
                f = stash[("f", ci)]
                dxb = work.tile([96, T], BF16, tag="dxb")
                nc.vector.tensor_mul(dxb[:], f["dlb"][:], f["xcA"][:])
                wsrc2 = dxb[:].unsqueeze(1).broadcast_to([96, 16, T])
                wdst = dxscr.ap().rearrange("(d r) t -> d r t", r=16)
                nc.sync.dma_start(wdst, wsrc2)
                dx_rep = big.tile([128, NK * T], BF16, tag="dx_rep", bufs=2)
                rsrc = dxscr.ap().rearrange("(k p) t -> p k t", p=128)
                rdst = dx_rep[:].rearrange("p (k t) -> p k t", k=NK)
                nc.sync.dma_start(rdst, rsrc)
                brep_ps = ps_a.tile([128, T], FP32, tag="psa")
                nc.tensor.matmul(brep_ps[:], cst["selbc"][:, 0:128], f["dblb"][:])
                brep = work.tile([128, T], BF16, tag="brep")
                nc.scalar.copy(brep[:], brep_ps[:])
                crep_ps = ps_a.tile([128, T], FP32, tag="psa")
                nc.tensor.matmul(crep_ps[:], cst["selbc"][:, 128:256], f["dblb"][:])
                crep = work.tile([128, T], BF16, tag="crep", bufs=3)
                nc.scalar.copy(crep[:], crep_ps[:])
                f.update(dxb=dxb, dx_rep=dx_rep, brep=brep, crep=crep)

            def seg_bn(ci):
                f = stash[("f", ci)]
                bn_all = big.tile([128, NK * T], BF16, tag="bn_all", bufs=2)
                nc.vector.tensor_mul(
                    bn_all[:].rearrange("p (k t) -> p k t", k=NK),
                    f["dx_rep"][:].rearrange("p (k t) -> p k t", k=NK),
                    f["brep"][:].unsqueeze(1).broadcast_to([128, NK, T]),
                )
                f["bn"] = bn_all

            def seg_an(ci):
                f = stash[("f", ci)]
                an_all = big.tile([128, NK * T], FP32, tag="an_all", bufs=2)
                for k in range(NK):
                    dlrep = ps_a.tile([128, T], FP32, tag="psa")
                    nc.tensor.matmul(
                        dlrep[:], cst["seldl"][:, 128 * k:128 * (k + 1)],
                        f["dlb"][:])
                    nc.scalar.activation(
                        an_all[:, k * T:(k + 1) * T], dlrep[:], AF.Exp,
                        scale=cst["ascv"][:, k:k + 1])
                f["an"] = an_all

            def scans(ci, ks):
                f = stash[("f", ci)]
                hn_all = hn_tiles[ci % 2]
                hn_prev = hn_tiles[(ci + 1) % 2]
                for k in ks:
                    init = (0.0 if ci == 0
                            else hn_prev[:, (k + 1) * T - 1:(k + 1) * T])
                    nc.vector.tensor_tensor_scan(
                        hn_all[:, k * T:(k + 1) * T],
                        f["an"][:, k * T:(k + 1) * T],
                        f["bn"][:, k * T:(k + 1) * T], init, OP.mult, OP.add)

            def back_pn(ci, half):
                f = stash[("f", ci)]
                hn_all = hn_tiles[ci % 2]
                if half == 0:
                    f["pn"] = big.tile([128, NK * T], BF16, tag="pn_all",
                                       bufs=1, name="pn_all")
                pn, crep = f["pn"], f["crep"]
                h = NK // 2
                s = slice(half * h * T, (half + h * half + (1 - half) * h) * T)
                ks = range(half * h, half * h + h)
                nc.vector.tensor_mul(
                    pn[:, half * h * T:(half * h + h) * T].rearrange(
                        "p (k t) -> p k t", k=h),
                    hn_all[:, half * h * T:(half * h + h) * T].rearrange(
                        "p (k t) -> p k t", k=h),
                    crep[:].unsqueeze(1).broadcast_to([128, h, T]),
                )

            def back_yacc(ci, half):
                f = stash[("f", ci)]
                if half == 0:
                    yacc = ps_y.tile([96, T], FP32, tag="y")
                    nc.tensor.matmul(yacc[:], cst["dvm"][:], f["xcA"][:],
                                     start=True, stop=False)
                    f["yacc"] = yacc
                    ks = range(0, NK // 2)
                else:
                    yacc = f["yacc"]
                    ks = range(NK // 2, NK)
                for k in ks:
                    nc.tensor.matmul(
                        yacc[:], cst["sely"][:, 96 * k:96 * (k + 1)],
                        f["pn"][:, k * T:(k + 1) * T],
                        start=False, stop=(k == NK - 1))

            def back_y2(ci):
                f = stash[("f", ci)]
                y2 = work.tile([96, T], BF16, tag="y2")
                nc.vector.tensor_mul(y2[:], f["yacc"][:], f["zs"][:])
                f["y2"] = y2

            def back_out(ci):
                f = stash.pop(("f", ci))
                sl = bass.ts(ci, T)
                po = ps_proj.tile([96, T], FP32, tag="proj")
                nc.tensor.matmul(po[:], cst["wout"][:], f["y2"][:])
                pos = work.tile([96, T], BF16, tag="pos")
                nc.scalar.copy(pos[:], po[:])
                nc.gpsimd.dma_start(pout.ap()[:, sl], pos[:])

            seg_a(0)
            for ci in range(nchunk + 2):
                fr, bk = ci < nchunk, ci >= 2
                if bk:
                    scans(ci - 2, range(0, 4))
                if fr:
                    seg_var(ci)
                    seg_rstd(ci)
                if bk:
                    scans(ci - 2, range(4, 6))
                    back_pn(ci - 2, 0)
                if fr:
                    seg_xn(ci)
                    seg_conv(ci)
                if bk:
                    back_yacc(ci - 2, 0)
                    scans(ci - 2, range(6, 10))
                if fr:
                    seg_silu_mul(ci)
                    seg_proj(ci)
                if bk:
                    scans(ci - 2, range(10, NK))
                    back_pn(ci - 2, 1)
                    back_yacc(ci - 2, 1)
                if fr:
                    seg_dx(ci)
                    seg_bn(ci)
                if ci + 1 < nchunk:
                    seg_a(ci + 1)
                if fr:
                    seg_an(ci)
                if bk:
                    back_y2(ci - 2)
                    back_out(ci - 2)
    _split_waits(nc)
    return nc


# ---------------------------------------------------------------- host side
def _bf(x):
    import ml_dtypes
    return np.asarray(x, dtype=ml_dtypes.bfloat16)


def make_core_inputs(inputs, c, ltot=L):
    b, d, half = c // 4, (c // 2) % 2, c % 2
    hs = slice(half * 96, half * 96 + 96)
    oth = slice((1 - half) * 96, (1 - half) * 96 + 96)
    x = np.asarray(inputs["x"], np.float32)
    xb = x[b].reshape(CH, L)[:, :ltot]
    if d == 1:
        xb = xb[:, ::-1]
    pfx = "f_" if d == 0 else "b_"
    g = lambda n: np.asarray(inputs[pfx + n], np.float32)
    in_w = g("in_w")
    conv_w = g("conv_w")[:, 0, :]
    conv_b = g("conv_b")
    xproj_w = g("xproj_w")
    dt_w = g("dt_w")
    dt_b = g("dt_b")
    A = -np.exp(g("A_log"))
    D = g("D")
    out_w = np.asarray(inputs["out_w"], np.float32)
    gam = np.asarray(inputs["ln_g"], np.float32)
    bet = np.asarray(inputs["ln_b"], np.float32)

    # fused conv+in_proj lhsT [98, 8*96]: tap j of half X
    wcv = np.zeros((98, 8 * 96), np.float32)
    for hf, sel in ((0, hs), (1, oth)):
        Win = in_w[sel]                     # [96 d, 96 c]
        cw = conv_w[sel]                    # [96 d, 4]
        cb = conv_b[sel]
        Wb = Win @ bet                      # [96]
        Wg1 = Win @ gam * 0  # placeholder
        rowsum = (Win * gam[None, :]).sum(1)   # sum_c gamma_c Win[d,c]
        for j in range(4):
            col = (4 * hf + j) * 96
            wcv[0:96, col:col + 96] = (gam[:, None] * Win.T) * cw[:, j][None, :]
            wcv[97, col:col + 96] = cw[:, j] * Wb + (cb if j == 3 else 0.0)
            wcv[96, col:col + 96] = -cw[:, j] * rowsum / CH
    # z lhsT
    Wz = in_w[192 + half * 96:192 + half * 96 + 96]
    wz = np.zeros((98, 96), np.float32)
    wz[0:96] = gam[:, None] * Wz.T
    wz[97] = Wz @ bet
    wz[96] = -(Wz * gam[None, :]).sum(1) / CH

    # x_proj lhsT, K-split [96, 2*38]
    wxp = np.zeros((96, 2 * NBC), np.float32)
    wxp[:, 0:NBC] = xproj_w[:, hs].T
    wxp[:, NBC:] = xproj_w[:, oth].T

    wdt = dt_w[hs].T                        # [6, 96]
    dtb = dt_b[hs][:, None]

    # packed-layout selectors: p = 16*q + n, d = 8k + q
    seldl = np.zeros((96, NK * 128), np.float32)
    sely = np.zeros((128, NK * 96), np.float32)
    ascv = np.zeros((128, NK), np.float32)
    Ah = A[hs]                              # [96, 16]
    for k in range(NK):
        for p in range(128):
            q, n = p // 16, p % 16
            dloc = 8 * k + q
            seldl[dloc, 128 * k + p] = 1.0
            sely[p, 96 * k + dloc] = 1.0
            ascv[p, k] = Ah[dloc, n]
    selbc = np.zeros((NBC, 2 * 128), np.float32)
    for p in range(128):
        n = p % 16
        selbc[DTR + n, p] = 1.0
        selbc[DTR + NST + n, 128 + p] = 1.0

    return {
        "xin": _bf(xb),
        "wcv": _bf(wcv),
        "wz": _bf(wz),
        "wxp": _bf(wxp),
        "wdt": _bf(wdt),
        "dtb": np.ascontiguousarray(dtb, np.float32),
        "seldl": _bf(seldl),
        "selbc": _bf(selbc),
        "ascv": np.ascontiguousarray(ascv, np.float32),
        "dvm": _bf(np.diag(D[hs])),
        "sely": _bf(sely),
        "wout": _bf(out_w[:, hs].T),
        "ones1": _bf(np.ones((CH, 1))),
        "ones_r": _bf(np.ones((1, CH))),
    }, (b, d)


_CACHE = {}


def kernel(**inputs):
    if "nc" not in _CACHE:
        _CACHE["nc"] = build_program()
    nc = _CACHE["nc"]
    in_maps, metas = [], []
    for c in range(8):
        m, meta = make_core_inputs(inputs, c)
        in_maps.append(m)
        metas.append(meta)
    res = run_bass_kernel_spmd(nc, in_maps, list(range(8)))
    x = np.asarray(inputs["x"], np.float32)
    out = x.copy()
    for c in range(8):
        b, d = metas[c]
        po = np.asarray(res.results[c]["pout"], np.float32)
        if d == 1:
            po = po[:, ::-1]
        out[b] += po.reshape(CH, H, W)
    return out


# revision 3
# speedup vs baseline: 1.0547x; 1.0547x over previous
"""Bidirectional spatial Mamba block on 8 Trainium2 NeuronCores — v2.

Sharding: core c = b*4 + dir*2 + half handles batch b, scan direction dir
(backward cores get host-reversed input; host un-reverses their output),
and d-half `half` of the DIN=192 inner channels. Each core runs an identical
SPMD program producing a [96, L] partial of out_w @ y_dir; the host sums the
four partials per batch and adds the residual x.

v2 design (vs baseline):
- (d,n) packed scan: the 96 d-channels x 16 states = 1536 recurrence rows are
  packed into 12 tiles of 128 partitions (p = 16*q + n, d = 8k + q), so each
  chunk runs 12 tensor_tensor_scans of [128, T] instead of 16 of [96, T].
- bf16 everywhere except the scan decay path (delta kept fp32-relative).
- All matmuls bf16 (fp32 matmul is 2 instructions and ~3x slower).
- GpSimd left idle: Pool ops contend with DVE for the SBUF port and slow
  the scans down ~40%.
- LN affine, conv bias, and the -mu*rstd LN term are folded into the fused
  in_proj+conv matmul via two extra input rows (ones row, mrs row).
- B/C/dl broadcasts into the packed layout via PE selector matmuls;
  dx replication via a DRAM round-trip DMA (write-side 16x replication).
- y = sum_n C_n*h_n accumulated in PSUM by 12 selector matmuls + a diag(D)
  matmul (the D*xc term), so no elementwise adds are needed.
"""
import numpy as np

import concourse.bass as bass
import concourse.mybir as mybir
import concourse.tile as tile
from concourse.bass_utils import run_bass_kernel_spmd

AF = mybir.ActivationFunctionType
OP = mybir.AluOpType
FP32 = mybir.dt.float32
BF16 = mybir.dt.bfloat16

CH, DIN, NST, DTR, DCONV = 96, 192, 16, 6, 4
B, H, W = 2, 128, 128
L = H * W
T = 512
NK = 12          # (d,n) tiles of 128 partitions: 96*16 / 128
NBC = DTR + 2 * NST   # 38 x_proj rows


# ---------------------------------------------------------------- tile patch
# This walrus codegen rejects the multi-wait Drain that TileContext emits at
# exit ("Too many sync wait commands"); split the waits onto single-wait NoOps.
_PATCHED = False


def _patch_tile_drain():
    global _PATCHED
    if _PATCHED:
        return
    _PATCHED = True
    from bass_rust import ScopedClock

    def patched(self, tick_clock, wait_clock):
        nc = self.nc
        carrier = nc.sync.nop()
        wait_clock.add_sem_waits(
            carrier.ins, ScopedClock({None: tick_clock.global_clock})
        )
        si = carrier.ins.sync_info
        waits = list(si.on_wait) if si is not None else []
        if si is not None:
            si.on_wait = waits[:1]
            for w in waits[1:]:
                n2 = nc.sync.nop()
                n2.ins.sync_info = mybir.SyncInfo(on_wait=[w], on_update=[])
        nc.sync.drain()
        nc.all_engine_barrier()
        assert self.sems is not None
        popped = nc._tile_sem_poison_stack.pop()
        assert popped is self._sem_poison
        nc.clear_and_free_semaphores(list(self.sems.allocated().values()))
        nc.all_engine_barrier()

    tile.TileContext._drain_and_barrier = patched


def _split_waits(nc, max_waits=1):
    """Walrus rejects instructions carrying more than ~1 sem wait. Hoist
    extras onto same-engine NoOp carriers inserted just before."""
    for bb in nc.main_func.blocks:
        new_insts = []
        for ins in bb.instructions:
            si = ins.sync_info
            if si is not None and len(si.on_wait) > max_waits:
                waits = list(si.on_wait)
                for w in waits[max_waits:]:
                    nop = mybir.InstNoOp(
                        name=nc.get_next_instruction_name(),
                        engine=ins.engine, ins=[], outs=[],
                        sync_info=mybir.SyncInfo(on_wait=[w], on_update=[]),
                    )
                    nc.register_instruction(nop)
                    new_insts.append(nop)
                si.on_wait = waits[:max_waits]
            new_insts.append(ins)
        bb.instructions[:] = new_insts


# ---------------------------------------------------------------- builder
def build_program(nchunk=L // T):
    _patch_tile_drain()
    nc = bass.Bass(num_devices=8)
    nc.allow_non_contiguous_dma("broadcast/replication DMAs")
    # const AP so activation(bias=const) lowers
    eps_t = nc.alloc_sbuf_tensor("const-f32-lneps", [128, 1], FP32)
    nc.gpsimd.memset(eps_t.ap(), 1e-5)
    nc.const_aps.aps[(FP32, 1e-5)] = eps_t.ap()
    one_t = nc.alloc_sbuf_tensor("const-f32-one", [128, 1], FP32)
    nc.gpsimd.memset(one_t.ap(), 1.0)
    nc.const_aps.aps[(FP32, 1.0)] = one_t.ap()
    nc.all_engine_barrier()
    Ltot = nchunk * T

    din = {}
    for name, shape, dt in [
        ("xin", [CH, Ltot], BF16),
        ("wcv", [98, 8 * 96], BF16),      # fused in_proj+conv lhsT, 4 taps x 2 halves
        ("wz", [98, 96], BF16),
        ("wxp", [96, 2 * NBC], BF16),     # x_proj lhsT, K-split halves
        ("wdt", [DTR, 96], BF16),
        ("dtb", [CH, 1], FP32),
        ("seldl", [96, NK * 128], BF16),  # d -> packed(p) replication
        ("selbc", [NBC, 2 * 128], BF16),  # B,C row -> packed(p)
        ("ascv", [128, NK], FP32),        # A[d(p,k), n(p)]
        ("dvm", [96, 96], BF16),          # diag(D)
        ("sely", [128, NK * 96], BF16),   # packed(p) -> d contraction
        ("wout", [96, 96], BF16),
        ("ones1", [CH, 1], BF16),
        ("ones_r", [1, CH], BF16),
    ]:
        din[name] = nc.declare_dram_parameter(name, shape, dt, isOutput=False)
    pout = nc.declare_dram_parameter("pout", [CH, Ltot], BF16, isOutput=True)
    # dram scratch for dx replication round-trip (write-side 16x replication)
    dxscr = nc.declare_dram_parameter("dxscr", [NK * 128, T], BF16, isOutput=True)

    with tile.TileContext(nc) as tc:
        with (
            tc.tile_pool(name="const", bufs=1) as const,
            tc.tile_pool(name="io", bufs=3) as io,
            tc.tile_pool(name="work", bufs=2) as work,
            tc.tile_pool(name="small", bufs=2) as small,
            tc.tile_pool(name="big", bufs=2) as big,
            tc.tile_pool(name="ps_a", bufs=2, space="PSUM") as ps_a,
            tc.tile_pool(name="ps_mm", bufs=2, space="PSUM") as ps_mm,
            tc.tile_pool(name="ps_proj", bufs=2, space="PSUM") as ps_proj,
            tc.tile_pool(name="ps_po", bufs=1, space="PSUM") as ps_po,
            tc.tile_pool(name="ps_y", bufs=1, space="PSUM") as ps_y,
        ):
            cst = {}
            for name in ["wcv", "wz", "wxp", "wdt", "dtb", "seldl", "selbc",
                         "ascv", "dvm", "sely", "wout", "ones1", "ones_r"]:
                t = const.tile(list(din[name].shape), din[name].dtype,
                               tag=name, name=name)
                nc.gpsimd.dma_start(t[:], din[name].ap()[:])
                cst[name] = t

            # persistent ping-pong xn tiles [98, T+3]: rows 0:96 = xhat,
            # row 96 = mu*sum(gam*W) fold (mrs), row 97 = ones (bias fold)
            xn_tiles = [const.tile([98, T + 3], BF16, tag=f"xn{i}",
                                   name=f"xn{i}") for i in range(2)]
            for t_ in xn_tiles:
                nc.vector.memset(t_[:], 0.0)
                nc.vector.memset(t_[96:98, 3:T + 3], 1.0)
            # hn ping-pong [128, NK*T]
            hn_tiles = [const.tile([128, NK * T], BF16, tag=f"hn{i}",
                                   name=f"hn{i}") for i in range(2)]
            nc.vector.memset(hn_tiles[1][:], 0.0)

            # cross-stage state; emission order IS dependency order in Tile,
            # so every consumer is emitted after its producer. Front ops of
            # chunk ci are woven between the scans of chunk ci-2 such that
            # each engine stream rarely waits.
            stash = {}

            def seg_a(ci):
                """LN head: input DMA + sums (Act/PE)."""
                sl = bass.ts(ci, T)
                xt = io.tile([CH, T], BF16, tag="xt")
                nc.gpsimd.dma_start(xt[:], din["xin"].ap()[:, sl])
                xsq = work.tile([CH, T], BF16, tag="xsq")
                nc.scalar.square(xsq[:], xt[:])
                s1 = ps_a.tile([1, T], FP32, tag="psa")
                nc.tensor.matmul(s1[:], cst["ones1"][:], xt[:])
                musq = small.tile([1, T], FP32, tag="musq")
                nc.scalar.activation(musq[:], s1[:], AF.Square, scale=1.0 / CH)
                s2 = ps_a.tile([1, T], FP32, tag="psa")
                nc.tensor.matmul(s2[:], cst["ones1"][:], xsq[:])
                stash[("f", ci)] = f = {}
                f.update(xt=xt, s1=s1, s2=s2, musq=musq)

            def seg_var(ci):
                f = stash[("f", ci)]
                var = small.tile([1, T], FP32, tag="var")
                nc.vector.scalar_tensor_tensor(
                    var[:], f["s2"][:], 1.0 / CH, f["musq"][:],
                    OP.mult, OP.subtract)
                f["var"] = var

            def seg_rstd(ci):
                f = stash[("f", ci)]
                lv = small.tile([1, T], FP32, tag="lv")
                nc.scalar.activation(lv[:], f["var"][:], AF.Ln, bias=1e-5)
                rstd = small.tile([1, T], BF16, tag="rstd")
                nc.scalar.activation(rstd[:], lv[:], AF.Exp, scale=-0.5)
                rstd_b = ps_a.tile([96, T], FP32, tag="psa")
                nc.tensor.matmul(rstd_b[:], cst["ones_r"][:], rstd[:])
                f.update(rstd=rstd, rstd_b=rstd_b)

            def seg_xn(ci):
                """mrs row, xhat rows, halo (DVE)."""
                f = stash[("f", ci)]
                xn = xn_tiles[ci % 2]
                xn_nxt = xn_tiles[(ci + 1) % 2]
                nc.vector.tensor_mul(xn[96:97, 3:T + 3], f["s1"][:], f["rstd"][:])
                nc.vector.tensor_mul(xn[0:96, 3:T + 3], f["xt"][:], f["rstd_b"][:])
                nc.vector.tensor_copy(xn_nxt[:, 0:3], xn[:, T:T + 3])

            def seg_conv(ci):
                """conv/z matmuls + the three sigmoid chains (PE/Act)."""
                f = stash[("f", ci)]
                xn = xn_tiles[ci % 2]
                cps = []
                for hf in range(2):
                    cp = ps_mm.tile([96, T], FP32, tag="mm")
                    for j in range(4):
                        nc.tensor.matmul(
                            cp[:],
                            cst["wcv"][:, (4 * hf + j) * 96:(4 * hf + j + 1) * 96],
                            xn[:, j:j + T],
                            start=(j == 0), stop=(j == 3))
                    cps.append(cp)
                zps = ps_mm.tile([96, T], FP32, tag="mm")
                nc.tensor.matmul(zps[:], cst["wz"][:], xn[:, 3:T + 3])
                sgs = []
                for idx, ps in enumerate(cps + [zps]):
                    en1 = work.tile([96, T], FP32, tag="en1",
                                    name=f"en1_{idx}", bufs=2)
                    nc.scalar.activation(en1[:], ps[:], AF.Exp, scale=-1.0)
                    nc.scalar.activation(en1[:], en1[:], AF.Ln, bias=1.0)
                    sg = work.tile([96, T], BF16, tag="sg",
                                   name=f"sg_{idx}", bufs=3)
                    nc.scalar.activation(sg[:], en1[:], AF.Exp, scale=-1.0)
                    sgs.append(sg)
                f.update(cps=cps, zps=zps, sgs=sgs)

            def seg_silu_mul(ci):
                f = stash[("f", ci)]
                xcA = work.tile([96, T], BF16, tag="xcA", bufs=3)
                xcB = work.tile([96, T], BF16, tag="xcB", bufs=2)
                zs = work.tile([96, T], BF16, tag="zs", bufs=3)
                nc.vector.tensor_mul(xcA[:], f["cps"][0][:], f["sgs"][0][:])
                nc.vector.tensor_mul(xcB[:], f["cps"][1][:], f["sgs"][1][:])
                nc.vector.tensor_mul(zs[:], f["zps"][:], f["sgs"][2][:])
                f.update(xcA=xcA, xcB=xcB, zs=zs)

            def seg_proj(ci):
                """x_proj + delta chain (PE/Act)."""
                f = stash[("f", ci)]
                dbl = ps_proj.tile([NBC, T], FP32, tag="proj")
                nc.tensor.matmul(dbl[:], cst["wxp"][:, 0:NBC], f["xcA"][:],
                                 start=True, stop=False)
                nc.tensor.matmul(dbl[:], cst["wxp"][:, NBC:2 * NBC], f["xcB"][:],
                                 start=False, stop=True)
                dblb = work.tile([NBC, T], BF16, tag="dblb")
                nc.scalar.copy(dblb[:], dbl[:])
                dpre = ps_proj.tile([96, T], FP32, tag="proj")
                nc.tensor.matmul(dpre[:], cst["wdt"][:], dblb[0:DTR, :])
                spe = work.tile([96, T], FP32, tag="spe")
                nc.scalar.activation(spe[:], dpre[:], AF.Exp, bias=cst["dtb"][:])
                dlb = work.tile([96, T], BF16, tag="dlb")
                nc.scalar.activation(dlb[:], spe[:], AF.Ln, bias=1.0)
                f.update(dblb=dblb, dlb=dlb)

            def seg_dx(ci):
                """dx mul (DVE), replication DMAs, B/C selector matmuls."""
                f = stash[("f", ci)]
                dxb = work.tile([96, T], BF16, tag="dxb")
                nc.vector.tensor_mul(dxb[:], f["dlb"][:], f["xcA"][:])
                wsrc2 = dxb[:].unsqueeze(1).broadcast_to([96, 16, T])
                wdst = dxscr.ap().rearrange("(d r) t -> d r t", r=16)
                nc.sync.dma_start(wdst, wsrc2)
                dx_rep = big.tile([128, NK * T], BF16, tag="dx_rep", bufs=2)
                rsrc = dxscr.ap().rearrange("(k p) t -> p k t", p=128)
                rdst = dx_rep[:].rearrange("p (k t) -> p k t", k=NK)
                nc.sync.dma_start(rdst, rsrc)
                brep_ps = ps_a.tile([128, T], FP32, tag="psa")
                nc.tensor.matmul(brep_ps[:], cst["selbc"][:, 0:128], f["dblb"][:])
                brep = work.tile([128, T], BF16, tag="brep")
                nc.scalar.copy(brep[:], brep_ps[:])
                crep_ps = ps_a.tile([128, T], FP32, tag="psa")
                nc.tensor.matmul(crep_ps[:], cst["selbc"][:, 128:256], f["dblb"][:])
                crep = work.tile([128, T], BF16, tag="crep", bufs=3)
                nc.scalar.copy(crep[:], crep_ps[:])
                f.update(dxb=dxb, dx_rep=dx_rep, brep=brep, crep=crep)

            def seg_bn(ci):
                f = stash[("f", ci)]
                bn_all = big.tile([128, NK * T], BF16, tag="bn_all", bufs=2)
                nc.vector.tensor_mul(
                    bn_all[:].rearrange("p (k t) -> p k t", k=NK),
                    f["dx_rep"][:].rearrange("p (k t) -> p k t", k=NK),
                    f["brep"][:].unsqueeze(1).broadcast_to([128, NK, T]),
                )
                f["bn"] = bn_all

            def seg_an(ci):
                f = stash[("f", ci)]
                an_all = big.tile([128, NK * T], FP32, tag="an_all", bufs=2)
                for k in range(NK):
                    dlrep = ps_a.tile([128, T], FP32, tag="psa")
                    nc.tensor.matmul(
                        dlrep[:], cst["seldl"][:, 128 * k:128 * (k + 1)],
                        f["dlb"][:])
                    nc.scalar.activation(
                        an_all[:, k * T:(k + 1) * T], dlrep[:], AF.Exp,
                        scale=cst["ascv"][:, k:k + 1])
                f["an"] = an_all

            def scans(ci, ks):
                f = stash[("f", ci)]
                hn_all = hn_tiles[ci % 2]
                hn_prev = hn_tiles[(ci + 1) % 2]
                for k in ks:
                    init = (0.0 if ci == 0
                            else hn_prev[:, (k + 1) * T - 1:(k + 1) * T])
                    nc.vector.tensor_tensor_scan(
                        hn_all[:, k * T:(k + 1) * T],
                        f["an"][:, k * T:(k + 1) * T],
                        f["bn"][:, k * T:(k + 1) * T], init, OP.mult, OP.add)

            def back_pn(ci, half):
                f = stash[("f", ci)]
                hn_all = hn_tiles[ci % 2]
                if half == 0:
                    f["pn"] = big.tile([128, NK * T], BF16, tag="pn_all",
                                       bufs=1, name="pn_all")
                pn, crep = f["pn"], f["crep"]
                h = NK // 2
                s = slice(half * h * T, (half + h * half + (1 - half) * h) * T)
                ks = range(half * h, half * h + h)
                nc.vector.tensor_mul(
                    pn[:, half * h * T:(half * h + h) * T].rearrange(
                        "p (k t) -> p k t", k=h),
                    hn_all[:, half * h * T:(half * h + h) * T].rearrange(
                        "p (k t) -> p k t", k=h),
                    crep[:].unsqueeze(1).broadcast_to([128, h, T]),
                )

            def back_yacc(ci, half):
                f = stash[("f", ci)]
                if half == 0:
                    yacc = ps_y.tile([96, T], FP32, tag="y")
                    nc.tensor.matmul(yacc[:], cst["dvm"][:], f["xcA"][:],
                                     start=True, stop=False)
                    f["yacc"] = yacc
                    ks = range(0, NK // 2)
                else:
                    yacc = f["yacc"]
                    ks = range(NK // 2, NK)
                for k in ks:
                    nc.tensor.matmul(
                        yacc[:], cst["sely"][:, 96 * k:96 * (k + 1)],
                        f["pn"][:, k * T:(k + 1) * T],
                        start=False, stop=(k == NK - 1))

            def back_y2(ci):
                f = stash[("f", ci)]
                y2 = work.tile([96, T], BF16, tag="y2")
                nc.vector.tensor_mul(y2[:], f["yacc"][:], f["zs"][:])
                f["y2"] = y2

            def back_out(ci):
                f = stash.pop(("f", ci))
                sl = bass.ts(ci, T)
                po = ps_proj.tile([96, T], FP32, tag="proj")
                nc.tensor.matmul(po[:], cst["wout"][:], f["y2"][:])
                pos = work.tile([96, T], BF16, tag="pos")
                nc.scalar.copy(pos[:], po[:])
                nc.gpsimd.dma_start(pout.ap()[:, sl], pos[:])

            seg_a(0)
            for ci in range(nchunk + 2):
                f1 = ci < nchunk          # front part 1 of chunk ci
                f2 = 0 <= ci - 1 < nchunk  # front part 2 of chunk ci-1
                bk = ci >= 2               # back phase of chunk ci-2
                if f2:
                    seg_proj(ci - 1)
                if bk:
                    scans(ci - 2, range(0, 4))
                if f1:
                    seg_var(ci)
                    seg_rstd(ci)
                if bk:
                    scans(ci - 2, range(4, 6))
                    back_pn(ci - 2, 0)
                if f2:
                    seg_dx(ci - 1)
                if bk:
                    back_yacc(ci - 2, 0)
                if f1:
                    seg_xn(ci)
                    seg_conv(ci)
                if bk:
                    scans(ci - 2, range(6, 10))
                if f2:
                    seg_bn(ci - 1)
                if bk:
                    scans(ci - 2, range(10, NK))
                    back_pn(ci - 2, 1)
                    back_yacc(ci - 2, 1)
                if f1:
                    seg_silu_mul(ci)
                if ci + 1 < nchunk:
                    seg_a(ci + 1)
                if f2:
                    seg_an(ci - 1)
                if bk:
                    back_y2(ci - 2)
                    back_out(ci - 2)
    _split_waits(nc)
    return nc


# ---------------------------------------------------------------- host side
def _bf(x):
    import ml_dtypes
    return np.asarray(x, dtype=ml_dtypes.bfloat16)


def make_core_inputs(inputs, c, ltot=L):
    b, d, half = c // 4, (c // 2) % 2, c % 2
    hs = slice(half * 96, half * 96 + 96)
    oth = slice((1 - half) * 96, (1 - half) * 96 + 96)
    x = np.asarray(inputs["x"], np.float32)
    xb = x[b].reshape(CH, L)[:, :ltot]
    if d == 1:
        xb = xb[:, ::-1]
    pfx = "f_" if d == 0 else "b_"
    g = lambda n: np.asarray(inputs[pfx + n], np.float32)
    in_w = g("in_w")
    conv_w = g("conv_w")[:, 0, :]
    conv_b = g("conv_b")
    xproj_w = g("xproj_w")
    dt_w = g("dt_w")
    dt_b = g("dt_b")
    A = -np.exp(g("A_log"))
    D = g("D")
    out_w = np.asarray(inputs["out_w"], np.float32)
    gam = np.asarray(inputs["ln_g"], np.float32)
    bet = np.asarray(inputs["ln_b"], np.float32)

    # fused conv+in_proj lhsT [98, 8*96]: tap j of half X
    wcv = np.zeros((98, 8 * 96), np.float32)
    for hf, sel in ((0, hs), (1, oth)):
        Win = in_w[sel]                     # [96 d, 96 c]
        cw = conv_w[sel]                    # [96 d, 4]
        cb = conv_b[sel]
        Wb = Win @ bet                      # [96]
        Wg1 = Win @ gam * 0  # placeholder
        rowsum = (Win * gam[None, :]).sum(1)   # sum_c gamma_c Win[d,c]
        for j in range(4):
            col = (4 * hf + j) * 96
            wcv[0:96, col:col + 96] = (gam[:, None] * Win.T) * cw[:, j][None, :]
            wcv[97, col:col + 96] = cw[:, j] * Wb + (cb if j == 3 else 0.0)
            wcv[96, col:col + 96] = -cw[:, j] * rowsum / CH
    # z lhsT
    Wz = in_w[192 + half * 96:192 + half * 96 + 96]
    wz = np.zeros((98, 96), np.float32)
    wz[0:96] = gam[:, None] * Wz.T
    wz[97] = Wz @ bet
    wz[96] = -(Wz * gam[None, :]).sum(1) / CH

    # x_proj lhsT, K-split [96, 2*38]
    wxp = np.zeros((96, 2 * NBC), np.float32)
    wxp[:, 0:NBC] = xproj_w[:, hs].T
    wxp[:, NBC:] = xproj_w[:, oth].T

    wdt = dt_w[hs].T                        # [6, 96]
    dtb = dt_b[hs][:, None]

    # packed-layout selectors: p = 16*q + n, d = 8k + q
    seldl = np.zeros((96, NK * 128), np.float32)
    sely = np.zeros((128, NK * 96), np.float32)
    ascv = np.zeros((128, NK), np.float32)
    Ah = A[hs]                              # [96, 16]
    for k in range(NK):
        for p in range(128):
            q, n = p // 16, p % 16
            dloc = 8 * k + q
            seldl[dloc, 128 * k + p] = 1.0
            sely[p, 96 * k + dloc] = 1.0
            ascv[p, k] = Ah[dloc, n]
    selbc = np.zeros((NBC, 2 * 128), np.float32)
    for p in range(128):
        n = p % 16
        selbc[DTR + n, p] = 1.0
        selbc[DTR + NST + n, 128 + p] = 1.0

    return {
        "xin": _bf(xb),
        "wcv": _bf(wcv),
        "wz": _bf(wz),
        "wxp": _bf(wxp),
        "wdt": _bf(wdt),
        "dtb": np.ascontiguousarray(dtb, np.float32),
        "seldl": _bf(seldl),
        "selbc": _bf(selbc),
        "ascv": np.ascontiguousarray(ascv, np.float32),
        "dvm": _bf(np.diag(D[hs])),
        "sely": _bf(sely),
        "wout": _bf(out_w[:, hs].T),
        "ones1": _bf(np.ones((CH, 1))),
        "ones_r": _bf(np.ones((1, CH))),
    }, (b, d)


_CACHE = {}


def kernel(**inputs):
    if "nc" not in _CACHE:
        _CACHE["nc"] = build_program()
    nc = _CACHE["nc"]
    in_maps, metas = [], []
    for c in range(8):
        m, meta = make_core_inputs(inputs, c)
        in_maps.append(m)
        metas.append(meta)
    res = run_bass_kernel_spmd(nc, in_maps, list(range(8)))
    x = np.asarray(inputs["x"], np.float32)
    out = x.copy()
    for c in range(8):
        b, d = metas[c]
        po = np.asarray(res.results[c]["pout"], np.float32)
        if d == 1:
            po = po[:, ::-1]
        out[b] += po.reshape(CH, H, W)
    return out


# revision 4
# speedup vs baseline: 1.0553x; 1.0005x over previous
"""Bidirectional spatial Mamba block on 8 Trainium2 NeuronCores — v2.

Sharding: core c = b*4 + dir*2 + half handles batch b, scan direction dir
(backward cores get host-reversed input; host un-reverses their output),
and d-half `half` of the DIN=192 inner channels. Each core runs an identical
SPMD program producing a [96, L] partial of out_w @ y_dir; the host sums the
four partials per batch and adds the residual x.

v2 design (vs baseline):
- (d,n) packed scan: the 96 d-channels x 16 states = 1536 recurrence rows are
  packed into 12 tiles of 128 partitions (p = 16*q + n, d = 8k + q), so each
  chunk runs 12 tensor_tensor_scans of [128, T] instead of 16 of [96, T].
- bf16 everywhere except the scan decay path (delta kept fp32-relative).
- All matmuls bf16 (fp32 matmul is 2 instructions and ~3x slower).
- GpSimd left idle: Pool ops contend with DVE for the SBUF port and slow
  the scans down ~40%.
- LN affine, conv bias, and the -mu*rstd LN term are folded into the fused
  in_proj+conv matmul via two extra input rows (ones row, mrs row).
- B/C/dl broadcasts into the packed layout via PE selector matmuls;
  dx replication via a DRAM round-trip DMA (write-side 16x replication).
- y = sum_n C_n*h_n accumulated in PSUM by 12 selector matmuls + a diag(D)
  matmul (the D*xc term), so no elementwise adds are needed.
"""
import numpy as np

import concourse.bass as bass
import concourse.mybir as mybir
import concourse.tile as tile
from concourse.bass_utils import run_bass_kernel_spmd

AF = mybir.ActivationFunctionType
OP = mybir.AluOpType
FP32 = mybir.dt.float32
BF16 = mybir.dt.bfloat16

CH, DIN, NST, DTR, DCONV = 96, 192, 16, 6, 4
B, H, W = 2, 128, 128
L = H * W
T = 512
NK = 12          # (d,n) tiles of 128 partitions: 96*16 / 128
NBC = DTR + 2 * NST   # 38 x_proj rows


# ---------------------------------------------------------------- tile patch
# This walrus codegen rejects the multi-wait Drain that TileContext emits at
# exit ("Too many sync wait commands"); split the waits onto single-wait NoOps.
_PATCHED = False


def _patch_tile_drain():
    global _PATCHED
    if _PATCHED:
        return
    _PATCHED = True
    from bass_rust import ScopedClock

    def patched(self, tick_clock, wait_clock):
        nc = self.nc
        carrier = nc.sync.nop()
        wait_clock.add_sem_waits(
            carrier.ins, ScopedClock({None: tick_clock.global_clock})
        )
        si = carrier.ins.sync_info
        waits = list(si.on_wait) if si is not None else []
        if si is not None:
            si.on_wait = waits[:1]
            for w in waits[1:]:
                n2 = nc.sync.nop()
                n2.ins.sync_info = mybir.SyncInfo(on_wait=[w], on_update=[])
        nc.sync.drain()
        nc.all_engine_barrier()
        assert self.sems is not None
        popped = nc._tile_sem_poison_stack.pop()
        assert popped is self._sem_poison
        nc.clear_and_free_semaphores(list(self.sems.allocated().values()))
        nc.all_engine_barrier()

    tile.TileContext._drain_and_barrier = patched


def _split_waits(nc, max_waits=1):
    """Walrus rejects instructions carrying more than ~1 sem wait. Hoist
    extras onto same-engine NoOp carriers inserted just before."""
    for bb in nc.main_func.blocks:
        new_insts = []
        for ins in bb.instructions:
            si = ins.sync_info
            if si is not None and len(si.on_wait) > max_waits:
                waits = list(si.on_wait)
                for w in waits[max_waits:]:
                    nop = mybir.InstNoOp(
                        name=nc.get_next_instruction_name(),
                        engine=ins.engine, ins=[], outs=[],
                        sync_info=mybir.SyncInfo(on_wait=[w], on_update=[]),
                    )
                    nc.register_instruction(nop)
                    new_insts.append(nop)
                si.on_wait = waits[:max_waits]
            new_insts.append(ins)
        bb.instructions[:] = new_insts


# ---------------------------------------------------------------- builder
def build_program(nchunk=L // T):
    _patch_tile_drain()
    nc = bass.Bass(num_devices=8)
    nc.allow_non_contiguous_dma("broadcast/replication DMAs")
    # const AP so activation(bias=const) lowers
    eps_t = nc.alloc_sbuf_tensor("const-f32-lneps", [128, 1], FP32)
    nc.gpsimd.memset(eps_t.ap(), 1e-5)
    nc.const_aps.aps[(FP32, 1e-5)] = eps_t.ap()
    one_t = nc.alloc_sbuf_tensor("const-f32-one", [128, 1], FP32)
    nc.gpsimd.memset(one_t.ap(), 1.0)
    nc.const_aps.aps[(FP32, 1.0)] = one_t.ap()
    nc.all_engine_barrier()
    Ltot = nchunk * T

    din = {}
    for name, shape, dt in [
        ("xin", [CH, Ltot], BF16),
        ("wcv", [98, 8 * 96], BF16),      # fused in_proj+conv lhsT, 4 taps x 2 halves
        ("wz", [98, 96], BF16),
        ("wxp", [96, 2 * NBC], BF16),     # x_proj lhsT, K-split halves
        ("wdt", [DTR, 96], BF16),
        ("dtb", [CH, 1], FP32),
        ("seldl", [96, NK * 128], BF16),  # d -> packed(p) replication
        ("selbc", [NBC, 2 * 128], BF16),  # B,C row -> packed(p)
        ("ascv", [128, NK], FP32),        # A[d(p,k), n(p)]
        ("dvm", [96, 96], BF16),          # diag(D)
        ("sely", [128, NK * 96], BF16),   # packed(p) -> d contraction
        ("wout", [96, 96], BF16),
        ("ones1", [CH, 1], BF16),
        ("ones_r", [1, CH], BF16),
    ]:
        din[name] = nc.declare_dram_parameter(name, shape, dt, isOutput=False)
    pout = nc.declare_dram_parameter("pout", [CH, Ltot], BF16, isOutput=True)
    # dram scratch for dx replication round-trip (write-side 16x replication)
    dxscr = nc.declare_dram_parameter("dxscr", [NK * 128, T], BF16, isOutput=True)

    with tile.TileContext(nc) as tc:
        with (
            tc.tile_pool(name="const", bufs=1) as const,
            tc.tile_pool(name="io", bufs=3) as io,
            tc.tile_pool(name="work", bufs=2) as work,
            tc.tile_pool(name="small", bufs=2) as small,
            tc.tile_pool(name="big", bufs=2) as big,
            tc.tile_pool(name="ps_a", bufs=2, space="PSUM") as ps_a,
            tc.tile_pool(name="ps_mm", bufs=2, space="PSUM") as ps_mm,
            tc.tile_pool(name="ps_proj", bufs=2, space="PSUM") as ps_proj,
            tc.tile_pool(name="ps_po", bufs=1, space="PSUM") as ps_po,
            tc.tile_pool(name="ps_y", bufs=1, space="PSUM") as ps_y,
        ):
            cst = {}
            for name in ["wcv", "wz", "wxp", "wdt", "dtb", "seldl", "selbc",
                         "ascv", "dvm", "sely", "wout", "ones1", "ones_r"]:
                t = const.tile(list(din[name].shape), din[name].dtype,
                               tag=name, name=name)
                nc.gpsimd.dma_start(t[:], din[name].ap()[:])
                cst[name] = t

            # persistent ping-pong xn tiles [98, T+3]: rows 0:96 = xhat,
            # row 96 = mu*sum(gam*W) fold (mrs), row 97 = ones (bias fold)
            xn_tiles = [const.tile([98, T + 3], BF16, tag=f"xn{i}",
                                   name=f"xn{i}") for i in range(2)]
            for t_ in xn_tiles:
                nc.vector.memset(t_[:], 0.0)
                nc.vector.memset(t_[96:98, 3:T + 3], 1.0)
            # hn ping-pong [128, NK*T]
            hn_tiles = [const.tile([128, NK * T], BF16, tag=f"hn{i}",
                                   name=f"hn{i}") for i in range(2)]
            nc.vector.memset(hn_tiles[1][:], 0.0)

            # cross-stage state; emission order IS dependency order in Tile,
            # so every consumer is emitted after its producer. Front ops of
            # chunk ci are woven between the scans of chunk ci-2 such that
            # each engine stream rarely waits.
            stash = {}

            def seg_a(ci):
                """LN head: input DMA + sums (Act/PE)."""
                sl = bass.ts(ci, T)
                xt = io.tile([CH, T], BF16, tag="xt")
                nc.gpsimd.dma_start(xt[:], din["xin"].ap()[:, sl])
                xsq = work.tile([CH, T], BF16, tag="xsq")
                nc.scalar.square(xsq[:], xt[:])
                s1 = ps_a.tile([1, T], FP32, tag="psa")
                nc.tensor.matmul(s1[:], cst["ones1"][:], xt[:])
                musq = small.tile([1, T], FP32, tag="musq")
                nc.scalar.activation(musq[:], s1[:], AF.Square, scale=1.0 / CH)
                s2 = ps_a.tile([1, T], FP32, tag="psa")
                nc.tensor.matmul(s2[:], cst["ones1"][:], xsq[:])
                stash[("f", ci)] = f = {}
                f.update(xt=xt, s1=s1, s2=s2, musq=musq)

            def seg_var(ci):
                f = stash[("f", ci)]
                var = small.tile([1, T], FP32, tag="var")
                nc.vector.scalar_tensor_tensor(
                    var[:], f["s2"][:], 1.0 / CH, f["musq"][:],
                    OP.mult, OP.subtract)
                f["var"] = var

            def seg_rstd(ci):
                f = stash[("f", ci)]
                lv = small.tile([1, T], FP32, tag="lv")
                nc.scalar.activation(lv[:], f["var"][:], AF.Ln, bias=1e-5)
                rstd = small.tile([1, T], BF16, tag="rstd")
                nc.scalar.activation(rstd[:], lv[:], AF.Exp, scale=-0.5)
                rstd_b = ps_a.tile([96, T], FP32, tag="psa")
                nc.tensor.matmul(rstd_b[:], cst["ones_r"][:], rstd[:])
                rstd_bc = work.tile([96, T], BF16, tag="rstd_bc")
                nc.scalar.copy(rstd_bc[:], rstd_b[:])
                f.update(rstd=rstd, rstd_bc=rstd_bc)

            def seg_xn(ci):
                """mrs row, xhat rows, halo (DVE)."""
                f = stash[("f", ci)]
                xn = xn_tiles[ci % 2]
                xn_nxt = xn_tiles[(ci + 1) % 2]
                nc.vector.tensor_mul(xn[96:97, 3:T + 3], f["s1"][:], f["rstd"][:])
                nc.vector.tensor_mul(xn[0:96, 3:T + 3], f["xt"][:], f["rstd_bc"][:])
                nc.vector.tensor_copy(xn_nxt[:, 0:3], xn[:, T:T + 3])

            def seg_conv(ci):
                """conv/z matmuls + the three sigmoid chains (PE/Act)."""
                f = stash[("f", ci)]
                xn = xn_tiles[ci % 2]
                cps = []
                for hf in range(2):
                    cp = ps_mm.tile([96, T], FP32, tag="mm")
                    for j in range(4):
                        nc.tensor.matmul(
                            cp[:],
                            cst["wcv"][:, (4 * hf + j) * 96:(4 * hf + j + 1) * 96],
                            xn[:, j:j + T],
                            start=(j == 0), stop=(j == 3))
                    cps.append(cp)
                zps = ps_mm.tile([96, T], FP32, tag="mm")
                nc.tensor.matmul(zps[:], cst["wz"][:], xn[:, 3:T + 3])
                sgs, xbs = [], []
                for idx, ps in enumerate(cps + [zps]):
                    en1 = work.tile([96, T], FP32, tag="en1",
                                    name=f"en1_{idx}", bufs=2)
                    nc.scalar.activation(en1[:], ps[:], AF.Exp, scale=-1.0)
                    nc.scalar.activation(en1[:], en1[:], AF.Ln, bias=1.0)
                    sg = work.tile([96, T], BF16, tag="sg",
                                   name=f"sg_{idx}", bufs=3)
                    nc.scalar.activation(sg[:], en1[:], AF.Exp, scale=-1.0)
                    xb = work.tile([96, T], BF16, tag="xb",
                                   name=f"xb_{idx}", bufs=3)
                    nc.scalar.copy(xb[:], ps[:])
                    sgs.append(sg)
                    xbs.append(xb)
                f.update(sgs=sgs, xbs=xbs)

            def seg_silu_mul(ci):
                f = stash[("f", ci)]
                xcA = work.tile([96, T], BF16, tag="xcA", bufs=3)
                xcB = work.tile([96, T], BF16, tag="xcB", bufs=2)
                zs = work.tile([96, T], BF16, tag="zs", bufs=3)
                nc.vector.tensor_mul(xcA[:], f["xbs"][0][:], f["sgs"][0][:])
                nc.vector.tensor_mul(xcB[:], f["xbs"][1][:], f["sgs"][1][:])
                nc.vector.tensor_mul(zs[:], f["xbs"][2][:], f["sgs"][2][:])
                f.update(xcA=xcA, xcB=xcB, zs=zs)

            def seg_proj(ci):
                """x_proj + delta chain (PE/Act)."""
                f = stash[("f", ci)]
                dbl = ps_proj.tile([NBC, T], FP32, tag="proj")
                nc.tensor.matmul(dbl[:], cst["wxp"][:, 0:NBC], f["xcA"][:],
                                 start=True, stop=False)
                nc.tensor.matmul(dbl[:], cst["wxp"][:, NBC:2 * NBC], f["xcB"][:],
                                 start=False, stop=True)
                dblb = work.tile([NBC, T], BF16, tag="dblb")
                nc.scalar.copy(dblb[:], dbl[:])
                dpre = ps_proj.tile([96, T], FP32, tag="proj")
                nc.tensor.matmul(dpre[:], cst["wdt"][:], dblb[0:DTR, :])
                spe = work.tile([96, T], FP32, tag="spe")
                nc.scalar.activation(spe[:], dpre[:], AF.Exp, bias=cst["dtb"][:])
                dlb = work.tile([96, T], BF16, tag="dlb")
                nc.scalar.activation(dlb[:], spe[:], AF.Ln, bias=1.0)
                f.update(dblb=dblb, dlb=dlb)

            def seg_dx(ci):
                """dx mul (DVE), replication DMAs, B/C selector matmuls."""
                f = stash[("f", ci)]
                dxb = work.tile([96, T], BF16, tag="dxb")
                nc.vector.tensor_mul(dxb[:], f["dlb"][:], f["xcA"][:])
                wsrc2 = dxb[:].unsqueeze(1).broadcast_to([96, 16, T])
                wdst = dxscr.ap().rearrange("(d r) t -> d r t", r=16)
                nc.sync.dma_start(wdst, wsrc2)
                dx_rep = big.tile([128, NK * T], BF16, tag="dx_rep", bufs=2)
                rsrc = dxscr.ap().rearrange("(k p) t -> p k t", p=128)
                rdst = dx_rep[:].rearrange("p (k t) -> p k t", k=NK)
                nc.sync.dma_start(rdst, rsrc)
                brep_ps = ps_a.tile([128, T], FP32, tag="psa")
                nc.tensor.matmul(brep_ps[:], cst["selbc"][:, 0:128], f["dblb"][:])
                brep = work.tile([128, T], BF16, tag="brep")
                nc.scalar.copy(brep[:], brep_ps[:])
                crep_ps = ps_a.tile([128, T], FP32, tag="psa")
                nc.tensor.matmul(crep_ps[:], cst["selbc"][:, 128:256], f["dblb"][:])
                crep = work.tile([128, T], BF16, tag="crep", bufs=3)
                nc.scalar.copy(crep[:], crep_ps[:])
                f.update(dxb=dxb, dx_rep=dx_rep, brep=brep, crep=crep)

            def seg_bn(ci):
                f = stash[("f", ci)]
                bn_all = big.tile([128, NK * T], BF16, tag="bn_all", bufs=2)
                nc.vector.tensor_mul(
                    bn_all[:].rearrange("p (k t) -> p k t", k=NK),
                    f["dx_rep"][:].rearrange("p (k t) -> p k t", k=NK),
                    f["brep"][:].unsqueeze(1).broadcast_to([128, NK, T]),
                )
                f["bn"] = bn_all

            def seg_an(ci):
                f = stash[("f", ci)]
                an_all = big.tile([128, NK * T], FP32, tag="an_all", bufs=2)
                for k in range(NK):
                    dlrep = ps_a.tile([128, T], FP32, tag="psa")
                    nc.tensor.matmul(
                        dlrep[:], cst["seldl"][:, 128 * k:128 * (k + 1)],
                        f["dlb"][:])
                    nc.scalar.activation(
                        an_all[:, k * T:(k + 1) * T], dlrep[:], AF.Exp,
                        scale=cst["ascv"][:, k:k + 1])
                f["an"] = an_all

            def scans(ci, ks):
                f = stash[("f", ci)]
                hn_all = hn_tiles[ci % 2]
                hn_prev = hn_tiles[(ci + 1) % 2]
                for k in ks:
                    init = (0.0 if ci == 0
                            else hn_prev[:, (k + 1) * T - 1:(k + 1) * T])
                    nc.vector.tensor_tensor_scan(
                        hn_all[:, k * T:(k + 1) * T],
                        f["an"][:, k * T:(k + 1) * T],
                        f["bn"][:, k * T:(k + 1) * T], init, OP.mult, OP.add)

            def back_pn(ci, half):
                f = stash[("f", ci)]
                hn_all = hn_tiles[ci % 2]
                if half == 0:
                    f["pn"] = big.tile([128, NK * T], BF16, tag="pn_all",
                                       bufs=1, name="pn_all")
                pn, crep = f["pn"], f["crep"]
                h = NK // 2
                s = slice(half * h * T, (half + h * half + (1 - half) * h) * T)
                ks = range(half * h, half * h + h)
                nc.vector.tensor_mul(
                    pn[:, half * h * T:(half * h + h) * T].rearrange(
                        "p (k t) -> p k t", k=h),
                    hn_all[:, half * h * T:(half * h + h) * T].rearrange(
                        "p (k t) -> p k t", k=h),
                    crep[:].unsqueeze(1).broadcast_to([128, h, T]),
                )

            def back_yacc(ci, half):
                f = stash[("f", ci)]
                if half == 0:
                    yacc = ps_y.tile([96, T], FP32, tag="y")
                    nc.tensor.matmul(yacc[:], cst["dvm"][:], f["xcA"][:],
                                     start=True, stop=False)
                    f["yacc"] = yacc
                    ks = range(0, NK // 2)
                else:
                    yacc = f["yacc"]
                    ks = range(NK // 2, NK)
                for k in ks:
                    nc.tensor.matmul(
                        yacc[:], cst["sely"][:, 96 * k:96 * (k + 1)],
                        f["pn"][:, k * T:(k + 1) * T],
                        start=False, stop=(k == NK - 1))

            def back_y2(ci):
                f = stash[("f", ci)]
                y2 = work.tile([96, T], BF16, tag="y2")
                nc.vector.tensor_mul(y2[:], f["yacc"][:], f["zs"][:])
                f["y2"] = y2

            def back_out(ci):
                f = stash.pop(("f", ci))
                sl = bass.ts(ci, T)
                po = ps_proj.tile([96, T], FP32, tag="proj")
                nc.tensor.matmul(po[:], cst["wout"][:], f["y2"][:])
                pos = work.tile([96, T], BF16, tag="pos")
                nc.scalar.copy(pos[:], po[:])
                nc.gpsimd.dma_start(pout.ap()[:, sl], pos[:])

            seg_a(0)
            for ci in range(nchunk + 2):
                f1 = ci < nchunk          # front part 1 of chunk ci
                f2 = 0 <= ci - 1 < nchunk  # front part 2 of chunk ci-1
                bk = ci >= 2               # back phase of chunk ci-2
                if f2:
                    seg_proj(ci - 1)
                if bk:
                    scans(ci - 2, range(0, 4))
                if f1:
                    seg_var(ci)
                    seg_rstd(ci)
                if bk:
                    scans(ci - 2, range(4, 6))
                    back_pn(ci - 2, 0)
                if f2:
                    seg_dx(ci - 1)
                if bk:
                    back_yacc(ci - 2, 0)
                if f1:
                    seg_xn(ci)
                    seg_conv(ci)
                if bk:
                    scans(ci - 2, range(6, 10))
                if f2:
                    seg_bn(ci - 1)
                if bk:
                    scans(ci - 2, range(10, NK))
                    back_pn(ci - 2, 1)
                    back_yacc(ci - 2, 1)
                if f1:
                    seg_silu_mul(ci)
                if ci + 1 < nchunk:
                    seg_a(ci + 1)
                if f2:
                    seg_an(ci - 1)
                if bk:
                    back_y2(ci - 2)
                    back_out(ci - 2)
    _split_waits(nc)
    return nc


# ---------------------------------------------------------------- host side
def _bf(x):
    import ml_dtypes
    return np.asarray(x, dtype=ml_dtypes.bfloat16)


def make_core_inputs(inputs, c, ltot=L):
    b, d, half = c // 4, (c // 2) % 2, c % 2
    hs = slice(half * 96, half * 96 + 96)
    oth = slice((1 - half) * 96, (1 - half) * 96 + 96)
    x = np.asarray(inputs["x"], np.float32)
    xb = x[b].reshape(CH, L)[:, :ltot]
    if d == 1:
        xb = xb[:, ::-1]
    pfx = "f_" if d == 0 else "b_"
    g = lambda n: np.asarray(inputs[pfx + n], np.float32)
    in_w = g("in_w")
    conv_w = g("conv_w")[:, 0, :]
    conv_b = g("conv_b")
    xproj_w = g("xproj_w")
    dt_w = g("dt_w")
    dt_b = g("dt_b")
    A = -np.exp(g("A_log"))
    D = g("D")
    out_w = np.asarray(inputs["out_w"], np.float32)
    gam = np.asarray(inputs["ln_g"], np.float32)
    bet = np.asarray(inputs["ln_b"], np.float32)

    # fused conv+in_proj lhsT [98, 8*96]: tap j of half X
    wcv = np.zeros((98, 8 * 96), np.float32)
    for hf, sel in ((0, hs), (1, oth)):
        Win = in_w[sel]                     # [96 d, 96 c]
        cw = conv_w[sel]                    # [96 d, 4]
        cb = conv_b[sel]
        Wb = Win @ bet                      # [96]
        Wg1 = Win @ gam * 0  # placeholder
        rowsum = (Win * gam[None, :]).sum(1)   # sum_c gamma_c Win[d,c]
        for j in range(4):
            col = (4 * hf + j) * 96
            wcv[0:96, col:col + 96] = (gam[:, None] * Win.T) * cw[:, j][None, :]
            wcv[97, col:col + 96] = cw[:, j] * Wb + (cb if j == 3 else 0.0)
            wcv[96, col:col + 96] = -cw[:, j] * rowsum / CH
    # z lhsT
    Wz = in_w[192 + half * 96:192 + half * 96 + 96]
    wz = np.zeros((98, 96), np.float32)
    wz[0:96] = gam[:, None] * Wz.T
    wz[97] = Wz @ bet
    wz[96] = -(Wz * gam[None, :]).sum(1) / CH

    # x_proj lhsT, K-split [96, 2*38]
    wxp = np.zeros((96, 2 * NBC), np.float32)
    wxp[:, 0:NBC] = xproj_w[:, hs].T
    wxp[:, NBC:] = xproj_w[:, oth].T

    wdt = dt_w[hs].T                        # [6, 96]
    dtb = dt_b[hs][:, None]

    # packed-layout selectors: p = 16*q + n, d = 8k + q
    seldl = np.zeros((96, NK * 128), np.float32)
    sely = np.zeros((128, NK * 96), np.float32)
    ascv = np.zeros((128, NK), np.float32)
    Ah = A[hs]                              # [96, 16]
    for k in range(NK):
        for p in range(128):
            q, n = p // 16, p % 16
            dloc = 8 * k + q
            seldl[dloc, 128 * k + p] = 1.0
            sely[p, 96 * k + dloc] = 1.0
            ascv[p, k] = Ah[dloc, n]
    selbc = np.zeros((NBC, 2 * 128), np.float32)
    for p in range(128):
        n = p % 16
        selbc[DTR + n, p] = 1.0
        selbc[DTR + NST + n, 128 + p] = 1.0

    return {
        "xin": _bf(xb),
        "wcv": _bf(wcv),
        "wz": _bf(wz),
        "wxp": _bf(wxp),
        "wdt": _bf(wdt),
        "dtb": np.ascontiguousarray(dtb, np.float32),
        "seldl": _bf(seldl),
        "selbc": _bf(selbc),
        "ascv": np.ascontiguousarray(ascv, np.float32),
        "dvm": _bf(np.diag(D[hs])),
        "sely": _bf(sely),
        "wout": _bf(out_w[:, hs].T),
        "ones1": _bf(np.ones((CH, 1))),
        "ones_r": _bf(np.ones((1, CH))),
    }, (b, d)


_CACHE = {}


def kernel(**inputs):
    if "nc" not in _CACHE:
        _CACHE["nc"] = build_program()
    nc = _CACHE["nc"]
    in_maps, metas = [], []
    for c in range(8):
        m, meta = make_core_inputs(inputs, c)
        in_maps.append(m)
        metas.append(meta)
    res = run_bass_kernel_spmd(nc, in_maps, list(range(8)))
    x = np.asarray(inputs["x"], np.float32)
    out = x.copy()
    for c in range(8):
        b, d = metas[c]
        po = np.asarray(res.results[c]["pout"], np.float32)
        if d == 1:
            po = po[:, ::-1]
        out[b] += po.reshape(CH, H, W)
    return out


# revision 5
# speedup vs baseline: 1.0760x; 1.0196x over previous
"""Bidirectional spatial Mamba block on 8 Trainium2 NeuronCores — v2.

Sharding: core c = b*4 + dir*2 + half handles batch b, scan direction dir
(backward cores get host-reversed input; host un-reverses their output),
and d-half `half` of the DIN=192 inner channels. Each core runs an identical
SPMD program producing a [96, L] partial of out_w @ y_dir; the host sums the
four partials per batch and adds the residual x.

v2 design (vs baseline):
- (d,n) packed scan: the 96 d-channels x 16 states = 1536 recurrence rows are
  packed into 12 tiles of 128 partitions (p = 16*q + n, d = 8k + q), so each
  chunk runs 12 tensor_tensor_scans of [128, T] instead of 16 of [96, T].
- bf16 everywhere except the scan decay path (delta kept fp32-relative).
- All matmuls bf16 (fp32 matmul is 2 instructions and ~3x slower).
- GpSimd left idle: Pool ops contend with DVE for the SBUF port and slow
  the scans down ~40%.
- LN affine, conv bias, and the -mu*rstd LN term are folded into the fused
  in_proj+conv matmul via two extra input rows (ones row, mrs row).
- B/C/dl broadcasts into the packed layout via PE selector matmuls;
  dx replication via a DRAM round-trip DMA (write-side 16x replication).
- y = sum_n C_n*h_n accumulated in PSUM by 12 selector matmuls + a diag(D)
  matmul (the D*xc term), so no elementwise adds are needed.
"""
import numpy as np

import concourse.bass as bass
import concourse.mybir as mybir
import concourse.tile as tile
from concourse.bass_utils import run_bass_kernel_spmd

AF = mybir.ActivationFunctionType
OP = mybir.AluOpType
FP32 = mybir.dt.float32
BF16 = mybir.dt.bfloat16

CH, DIN, NST, DTR, DCONV = 96, 192, 16, 6, 4
B, H, W = 2, 128, 128
L = H * W
T = 512
NK = 12          # (d,n) tiles of 128 partitions: 96*16 / 128
NBC = DTR + 2 * NST   # 38 x_proj rows


# ---------------------------------------------------------------- tile patch
# This walrus codegen rejects the multi-wait Drain that TileContext emits at
# exit ("Too many sync wait commands"); split the waits onto single-wait NoOps.
_PATCHED = False


def _patch_tile_drain():
    global _PATCHED
    if _PATCHED:
        return
    _PATCHED = True
    from bass_rust import ScopedClock

    def patched(self, tick_clock, wait_clock):
        nc = self.nc
        carrier = nc.sync.nop()
        wait_clock.add_sem_waits(
            carrier.ins, ScopedClock({None: tick_clock.global_clock})
        )
        si = carrier.ins.sync_info
        waits = list(si.on_wait) if si is not None else []
        if si is not None:
            si.on_wait = waits[:1]
            for w in waits[1:]:
                n2 = nc.sync.nop()
                n2.ins.sync_info = mybir.SyncInfo(on_wait=[w], on_update=[])
        nc.sync.drain()
        nc.all_engine_barrier()
        assert self.sems is not None
        popped = nc._tile_sem_poison_stack.pop()
        assert popped is self._sem_poison
        nc.clear_and_free_semaphores(list(self.sems.allocated().values()))
        nc.all_engine_barrier()

    tile.TileContext._drain_and_barrier = patched


def _split_waits(nc, max_waits=1):
    """Walrus rejects instructions carrying more than ~1 sem wait. Hoist
    extras onto same-engine NoOp carriers inserted just before."""
    for bb in nc.main_func.blocks:
        new_insts = []
        for ins in bb.instructions:
            si = ins.sync_info
            if si is not None and len(si.on_wait) > max_waits:
                waits = list(si.on_wait)
                for w in waits[max_waits:]:
                    nop = mybir.InstNoOp(
                        name=nc.get_next_instruction_name(),
                        engine=ins.engine, ins=[], outs=[],
                        sync_info=mybir.SyncInfo(on_wait=[w], on_update=[]),
                    )
                    nc.register_instruction(nop)
                    new_insts.append(nop)
                si.on_wait = waits[:max_waits]
            new_insts.append(ins)
        bb.instructions[:] = new_insts


# ---------------------------------------------------------------- builder
def build_program(nchunk=L // T):
    _patch_tile_drain()
    nc = bass.Bass(num_devices=8)
    nc.allow_non_contiguous_dma("broadcast/replication DMAs")
    # const AP so activation(bias=const) lowers
    eps_t = nc.alloc_sbuf_tensor("const-f32-lneps", [128, 1], FP32)
    nc.gpsimd.memset(eps_t.ap(), 1e-5)
    nc.const_aps.aps[(FP32, 1e-5)] = eps_t.ap()
    one_t = nc.alloc_sbuf_tensor("const-f32-one", [128, 1], FP32)
    nc.gpsimd.memset(one_t.ap(), 1.0)
    nc.const_aps.aps[(FP32, 1.0)] = one_t.ap()
    nc.all_engine_barrier()
    Ltot = nchunk * T

    din = {}
    for name, shape, dt in [
        ("xin", [CH, Ltot], BF16),
        ("wcv", [98, 8 * 96], BF16),      # fused in_proj+conv lhsT, 4 taps x 2 halves
        ("wz", [98, 96], BF16),
        ("wxp", [96, 2 * NBC], BF16),     # x_proj lhsT, K-split halves
        ("wdt", [DTR, 96], BF16),
        ("dtb", [CH, 1], FP32),
        ("seldl", [96, NK * 128], BF16),  # d -> packed(p) replication
        ("selbc", [NBC, 2 * 128], BF16),  # B,C row -> packed(p)
        ("ascv", [128, NK], FP32),        # A[d(p,k), n(p)]
        ("dvm", [96, 96], BF16),          # diag(D)
        ("sely", [128, NK * 96], BF16),   # packed(p) -> d contraction
        ("wout", [96, 96], BF16),
        ("ones1", [CH, 1], BF16),
        ("ones_r", [1, CH], BF16),
    ]:
        din[name] = nc.declare_dram_parameter(name, shape, dt, isOutput=False)
    pout = nc.declare_dram_parameter("pout", [CH, Ltot], BF16, isOutput=True)
    # dram scratch for dx replication round-trip (write-side 16x replication)
    dxscr = nc.declare_dram_parameter("dxscr", [NK * 128, T], BF16, isOutput=True)

    with tile.TileContext(nc) as tc:
        with (
            tc.tile_pool(name="const", bufs=1) as const,
            tc.tile_pool(name="io", bufs=3) as io,
            tc.tile_pool(name="work", bufs=2) as work,
            tc.tile_pool(name="small", bufs=2) as small,
            tc.tile_pool(name="big", bufs=2) as big,
            tc.tile_pool(name="ps_a", bufs=2, space="PSUM") as ps_a,
            tc.tile_pool(name="ps_mm", bufs=2, space="PSUM") as ps_mm,
            tc.tile_pool(name="ps_proj", bufs=2, space="PSUM") as ps_proj,
            tc.tile_pool(name="ps_po", bufs=1, space="PSUM") as ps_po,
            tc.tile_pool(name="ps_y", bufs=1, space="PSUM") as ps_y,
        ):
            cst = {}
            for name in ["wcv", "wz", "wxp", "wdt", "dtb", "seldl", "selbc",
                         "ascv", "dvm", "sely", "wout", "ones1", "ones_r"]:
                t = const.tile(list(din[name].shape), din[name].dtype,
                               tag=name, name=name)
                nc.gpsimd.dma_start(t[:], din[name].ap()[:])
                cst[name] = t

            # persistent ping-pong xn tiles [98, T+3]: rows 0:96 = xhat,
            # row 96 = mu*sum(gam*W) fold (mrs), row 97 = ones (bias fold)
            xn_tiles = [const.tile([98, T + 3], BF16, tag=f"xn{i}",
                                   name=f"xn{i}") for i in range(2)]
            for t_ in xn_tiles:
                nc.vector.memset(t_[:], 0.0)
                nc.vector.memset(t_[96:98, 3:T + 3], 1.0)
            # hn ping-pong [128, NK*T]
            hn_tiles = [const.tile([128, NK * T], BF16, tag=f"hn{i}",
                                   name=f"hn{i}") for i in range(2)]
            nc.vector.memset(hn_tiles[1][:], 0.0)

            # cross-stage state; emission order IS dependency order in Tile,
            # so every consumer is emitted after its producer. Front ops of
            # chunk ci are woven between the scans of chunk ci-2 such that
            # each engine stream rarely waits.
            stash = {}

            def seg_a(ci):
                """LN head: input DMA + sums (Act/PE)."""
                sl = bass.ts(ci, T)
                xt = io.tile([CH, T], BF16, tag="xt")
                nc.gpsimd.dma_start(xt[:], din["xin"].ap()[:, sl])
                xsq = work.tile([CH, T], BF16, tag="xsq")
                nc.scalar.square(xsq[:], xt[:])
                s1 = ps_a.tile([1, T], FP32, tag="psa")
                nc.tensor.matmul(s1[:], cst["ones1"][:], xt[:])
                musq = small.tile([1, T], FP32, tag="musq")
                nc.scalar.activation(musq[:], s1[:], AF.Square, scale=1.0 / CH)
                s2 = ps_a.tile([1, T], FP32, tag="psa")
                nc.tensor.matmul(s2[:], cst["ones1"][:], xsq[:])
                stash[("f", ci)] = f = {}
                f.update(xt=xt, s1=s1, s2=s2, musq=musq)

            def seg_var(ci):
                f = stash[("f", ci)]
                var = small.tile([1, T], FP32, tag="var")
                nc.vector.scalar_tensor_tensor(
                    var[:], f["s2"][:], 1.0 / CH, f["musq"][:],
                    OP.mult, OP.subtract)
                f["var"] = var

            def seg_rstd(ci):
                f = stash[("f", ci)]
                lv = small.tile([1, T], FP32, tag="lv")
                nc.scalar.activation(lv[:], f["var"][:], AF.Ln, bias=1e-5)
                rstd = small.tile([1, T], BF16, tag="rstd")
                nc.scalar.activation(rstd[:], lv[:], AF.Exp, scale=-0.5)
                rstd_b = ps_a.tile([96, T], FP32, tag="psa")
                nc.tensor.matmul(rstd_b[:], cst["ones_r"][:], rstd[:])
                rstd_bc = work.tile([96, T], BF16, tag="rstd_bc")
                nc.scalar.copy(rstd_bc[:], rstd_b[:])
                f.update(rstd=rstd, rstd_bc=rstd_bc)

            def seg_xn(ci):
                """mrs row, xhat rows, halo (DVE)."""
                f = stash[("f", ci)]
                xn = xn_tiles[ci % 2]
                xn_nxt = xn_tiles[(ci + 1) % 2]
                nc.vector.tensor_mul(xn[96:97, 3:T + 3], f["s1"][:], f["rstd"][:])
                nc.vector.tensor_mul(xn[0:96, 3:T + 3], f["xt"][:], f["rstd_bc"][:])
                nc.vector.tensor_copy(xn_nxt[:, 0:3], xn[:, T:T + 3])

            def seg_conv(ci):
                """conv/z matmuls + the three sigmoid chains (PE/Act)."""
                f = stash[("f", ci)]
                xn = xn_tiles[ci % 2]
                cps = []
                for hf in range(2):
                    cp = ps_mm.tile([96, T], FP32, tag="mm")
                    for j in range(4):
                        nc.tensor.matmul(
                            cp[:],
                            cst["wcv"][:, (4 * hf + j) * 96:(4 * hf + j + 1) * 96],
                            xn[:, j:j + T],
                            start=(j == 0), stop=(j == 3))
                    cps.append(cp)
                zps = ps_mm.tile([96, T], FP32, tag="mm")
                nc.tensor.matmul(zps[:], cst["wz"][:], xn[:, 3:T + 3])
                sgs, xbs = [], []
                for idx, ps in enumerate(cps + [zps]):
                    en1 = work.tile([96, T], FP32, tag="en1",
                                    name=f"en1_{idx}", bufs=2)
                    nc.scalar.activation(en1[:], ps[:], AF.Exp, scale=-1.0)
                    nc.scalar.activation(en1[:], en1[:], AF.Ln, bias=1.0)
                    sg = work.tile([96, T], BF16, tag="sg",
                                   name=f"sg_{idx}", bufs=3)
                    nc.scalar.activation(sg[:], en1[:], AF.Exp, scale=-1.0)
                    xb = work.tile([96, T], BF16, tag="xb",
                                   name=f"xb_{idx}", bufs=3)
                    nc.scalar.copy(xb[:], ps[:])
                    sgs.append(sg)
                    xbs.append(xb)
                f.update(sgs=sgs, xbs=xbs)

            def seg_silu_mul(ci):
                f = stash[("f", ci)]
                xcA = work.tile([96, T], BF16, tag="xcA", bufs=3)
                xcB = work.tile([96, T], BF16, tag="xcB", bufs=2)
                zs = work.tile([96, T], BF16, tag="zs", bufs=3)
                nc.vector.tensor_mul(xcA[:], f["xbs"][0][:], f["sgs"][0][:])
                nc.vector.tensor_mul(xcB[:], f["xbs"][1][:], f["sgs"][1][:])
                nc.vector.tensor_mul(zs[:], f["xbs"][2][:], f["sgs"][2][:])
                f.update(xcA=xcA, xcB=xcB, zs=zs)

            def seg_proj(ci):
                """x_proj + delta chain (PE/Act)."""
                f = stash[("f", ci)]
                dbl = ps_proj.tile([NBC, T], FP32, tag="proj")
                nc.tensor.matmul(dbl[:], cst["wxp"][:, 0:NBC], f["xcA"][:],
                                 start=True, stop=False)
                nc.tensor.matmul(dbl[:], cst["wxp"][:, NBC:2 * NBC], f["xcB"][:],
                                 start=False, stop=True)
                dblb = work.tile([NBC, T], BF16, tag="dblb")
                nc.scalar.copy(dblb[:], dbl[:])
                dpre = ps_proj.tile([96, T], FP32, tag="proj")
                nc.tensor.matmul(dpre[:], cst["wdt"][:], dblb[0:DTR, :])
                spe = work.tile([96, T], FP32, tag="spe")
                nc.scalar.activation(spe[:], dpre[:], AF.Exp, bias=cst["dtb"][:])
                dlb = work.tile([96, T], BF16, tag="dlb")
                nc.scalar.activation(dlb[:], spe[:], AF.Ln, bias=1.0)
                f.update(dblb=dblb, dlb=dlb)

            def seg_dx(ci):
                """dx mul (DVE), replication DMAs, B/C selector matmuls."""
                f = stash[("f", ci)]
                dxb = work.tile([96, T], BF16, tag="dxb")
                nc.vector.tensor_mul(dxb[:], f["dlb"][:], f["xcA"][:])
                wsrc2 = dxb[:].unsqueeze(1).broadcast_to([96, 16, T])
                wdst = dxscr.ap().rearrange("(d r) t -> d r t", r=16)
                nc.sync.dma_start(wdst, wsrc2)
                dx_rep = big.tile([128, NK * T], BF16, tag="dx_rep", bufs=2)
                rsrc = dxscr.ap().rearrange("(k p) t -> p k t", p=128)
                rdst = dx_rep[:].rearrange("p (k t) -> p k t", k=NK)
                nc.sync.dma_start(rdst, rsrc)
                brep_ps = ps_a.tile([128, T], FP32, tag="psa")
                nc.tensor.matmul(brep_ps[:], cst["selbc"][:, 0:128], f["dblb"][:])
                brep = work.tile([128, T], BF16, tag="brep")
                nc.scalar.copy(brep[:], brep_ps[:])
                crep_ps = ps_a.tile([128, T], FP32, tag="psa")
                nc.tensor.matmul(crep_ps[:], cst["selbc"][:, 128:256], f["dblb"][:])
                crep = work.tile([128, T], BF16, tag="crep", bufs=3)
                nc.scalar.copy(crep[:], crep_ps[:])
                f.update(dxb=dxb, dx_rep=dx_rep, brep=brep, crep=crep)

            def seg_bn(ci):
                f = stash[("f", ci)]
                bn_all = big.tile([128, NK * T], BF16, tag="bn_all", bufs=2)
                nc.vector.tensor_mul(
                    bn_all[:].rearrange("p (k t) -> p k t", k=NK),
                    f["dx_rep"][:].rearrange("p (k t) -> p k t", k=NK),
                    f["brep"][:].unsqueeze(1).broadcast_to([128, NK, T]),
                )
                f["bn"] = bn_all

            def seg_an(ci, ks):
                f = stash[("f", ci)]
                if "an" not in f:
                    f["an"] = big.tile([128, NK * T], FP32, tag="an_all",
                                       bufs=2, name="an_all")
                an_all = f["an"]
                for k in ks:
                    dlrep = ps_a.tile([128, T], FP32, tag="psa")
                    nc.tensor.matmul(
                        dlrep[:], cst["seldl"][:, 128 * k:128 * (k + 1)],
                        f["dlb"][:])
                    nc.scalar.activation(
                        an_all[:, k * T:(k + 1) * T], dlrep[:], AF.Exp,
                        scale=cst["ascv"][:, k:k + 1])

            def scans(ci, ks):
                f = stash[("f", ci)]
                hn_all = hn_tiles[ci % 2]
                hn_prev = hn_tiles[(ci + 1) % 2]
                for k in ks:
                    init = (0.0 if ci == 0
                            else hn_prev[:, (k + 1) * T - 1:(k + 1) * T])
                    nc.vector.tensor_tensor_scan(
                        hn_all[:, k * T:(k + 1) * T],
                        f["an"][:, k * T:(k + 1) * T],
                        f["bn"][:, k * T:(k + 1) * T], init, OP.mult, OP.add)

            def back_pn(ci, half):
                f = stash[("f", ci)]
                hn_all = hn_tiles[ci % 2]
                if half == 0:
                    f["pn"] = big.tile([128, NK * T], BF16, tag="pn_all",
                                       bufs=1, name="pn_all")
                pn, crep = f["pn"], f["crep"]
                h = NK // 2
                s = slice(half * h * T, (half + h * half + (1 - half) * h) * T)
                ks = range(half * h, half * h + h)
                nc.vector.tensor_mul(
                    pn[:, half * h * T:(half * h + h) * T].rearrange(
                        "p (k t) -> p k t", k=h),
                    hn_all[:, half * h * T:(half * h + h) * T].rearrange(
                        "p (k t) -> p k t", k=h),
                    crep[:].unsqueeze(1).broadcast_to([128, h, T]),
                )

            def back_yacc(ci, half):
                f = stash[("f", ci)]
                if half == 0:
                    yacc = ps_y.tile([96, T], FP32, tag="y")
                    nc.tensor.matmul(yacc[:], cst["dvm"][:], f["xcA"][:],
                                     start=True, stop=False)
                    f["yacc"] = yacc
                    ks = range(0, NK // 2)
                else:
                    yacc = f["yacc"]
                    ks = range(NK // 2, NK)
                for k in ks:
                    nc.tensor.matmul(
                        yacc[:], cst["sely"][:, 96 * k:96 * (k + 1)],
                        f["pn"][:, k * T:(k + 1) * T],
                        start=False, stop=(k == NK - 1))

            def back_y2(ci):
                f = stash[("f", ci)]
                y2 = work.tile([96, T], BF16, tag="y2")
                nc.vector.tensor_mul(y2[:], f["yacc"][:], f["zs"][:])
                f["y2"] = y2

            def back_out(ci):
                f = stash.pop(("f", ci))
                sl = bass.ts(ci, T)
                po = ps_proj.tile([96, T], FP32, tag="proj")
                nc.tensor.matmul(po[:], cst["wout"][:], f["y2"][:])
                pos = work.tile([96, T], BF16, tag="pos")
                nc.scalar.copy(pos[:], po[:])
                nc.gpsimd.dma_start(pout.ap()[:, sl], pos[:])

            seg_a(0)
            for ci in range(nchunk + 2):
                f1 = ci < nchunk          # front part 1 of chunk ci
                f2 = 0 <= ci - 1 < nchunk  # front part 2 of chunk ci-1
                bk = ci >= 2               # back phase of chunk ci-2
                if f2:
                    seg_proj(ci - 1)
                if bk:
                    scans(ci - 2, range(0, 4))
                if f1:
                    seg_var(ci)
                    seg_rstd(ci)
                if bk:
                    scans(ci - 2, range(4, 6))
                    back_pn(ci - 2, 0)
                if f2:
                    seg_dx(ci - 1)
                if bk:
                    back_yacc(ci - 2, 0)
                if f1:
                    seg_xn(ci)
                    seg_conv(ci)
                if bk:
                    scans(ci - 2, range(6, 10))
                    scans(ci - 2, range(10, NK))
                    back_pn(ci - 2, 1)
                    back_yacc(ci - 2, 1)
                if f2:
                    seg_bn(ci - 1)
                if f1:
                    seg_silu_mul(ci)
                if f2:
                    seg_an(ci - 1, range(0, 3))
                if ci + 1 < nchunk:
                    seg_a(ci + 1)
                if f2:
                    seg_an(ci - 1, range(3, NK))
                if bk:
                    back_y2(ci - 2)
                    back_out(ci - 2)
    _split_waits(nc)
    return nc


# ---------------------------------------------------------------- host side
def _bf(x):
    import ml_dtypes
    return np.asarray(x, dtype=ml_dtypes.bfloat16)


def make_core_inputs(inputs, c, ltot=L):
    b, d, half = c // 4, (c // 2) % 2, c % 2
    hs = slice(half * 96, half * 96 + 96)
    oth = slice((1 - half) * 96, (1 - half) * 96 + 96)
    x = np.asarray(inputs["x"], np.float32)
    xb = x[b].reshape(CH, L)[:, :ltot]
    if d == 1:
        xb = xb[:, ::-1]
    pfx = "f_" if d == 0 else "b_"
    g = lambda n: np.asarray(inputs[pfx + n], np.float32)
    in_w = g("in_w")
    conv_w = g("conv_w")[:, 0, :]
    conv_b = g("conv_b")
    xproj_w = g("xproj_w")
    dt_w = g("dt_w")
    dt_b = g("dt_b")
    A = -np.exp(g("A_log"))
    D = g("D")
    out_w = np.asarray(inputs["out_w"], np.float32)
    gam = np.asarray(inputs["ln_g"], np.float32)
    bet = np.asarray(inputs["ln_b"], np.float32)

    # fused conv+in_proj lhsT [98, 8*96]: tap j of half X
    wcv = np.zeros((98, 8 * 96), np.float32)
    for hf, sel in ((0, hs), (1, oth)):
        Win = in_w[sel]                     # [96 d, 96 c]
        cw = conv_w[sel]                    # [96 d, 4]
        cb = conv_b[sel]
        Wb = Win @ bet                      # [96]
        Wg1 = Win @ gam * 0  # placeholder
        rowsum = (Win * gam[None, :]).sum(1)   # sum_c gamma_c Win[d,c]
        for j in range(4):
            col = (4 * hf + j) * 96
            wcv[0:96, col:col + 96] = (gam[:, None] * Win.T) * cw[:, j][None, :]
            wcv[97, col:col + 96] = cw[:, j] * Wb + (cb if j == 3 else 0.0)
            wcv[96, col:col + 96] = -cw[:, j] * rowsum / CH
    # z lhsT
    Wz = in_w[192 + half * 96:192 + half * 96 + 96]
    wz = np.zeros((98, 96), np.float32)
    wz[0:96] = gam[:, None] * Wz.T
    wz[97] = Wz @ bet
    wz[96] = -(Wz * gam[None, :]).sum(1) / CH

    # x_proj lhsT, K-split [96, 2*38]
    wxp = np.zeros((96, 2 * NBC), np.float32)
    wxp[:, 0:NBC] = xproj_w[:, hs].T
    wxp[:, NBC:] = xproj_w[:, oth].T

    wdt = dt_w[hs].T                        # [6, 96]
    dtb = dt_b[hs][:, None]

    # packed-layout selectors: p = 16*q + n, d = 8k + q
    seldl = np.zeros((96, NK * 128), np.float32)
    sely = np.zeros((128, NK * 96), np.float32)
    ascv = np.zeros((128, NK), np.float32)
    Ah = A[hs]                              # [96, 16]
    for k in range(NK):
        for p in range(128):
            q, n = p // 16, p % 16
            dloc = 8 * k + q
            seldl[dloc, 128 * k + p] = 1.0
            sely[p, 96 * k + dloc] = 1.0
            ascv[p, k] = Ah[dloc, n]
    selbc = np.zeros((NBC, 2 * 128), np.float32)
    for p in range(128):
        n = p % 16
        selbc[DTR + n, p] = 1.0
        selbc[DTR + NST + n, 128 + p] = 1.0

    return {
        "xin": _bf(xb),
        "wcv": _bf(wcv),
        "wz": _bf(wz),
        "wxp": _bf(wxp),
        "wdt": _bf(wdt),
        "dtb": np.ascontiguousarray(dtb, np.float32),
        "seldl": _bf(seldl),
        "selbc": _bf(selbc),
        "ascv": np.ascontiguousarray(ascv, np.float32),
        "dvm": _bf(np.diag(D[hs])),
        "sely": _bf(sely),
        "wout": _bf(out_w[:, hs].T),
        "ones1": _bf(np.ones((CH, 1))),
        "ones_r": _bf(np.ones((1, CH))),
    }, (b, d)


_CACHE = {}


def kernel(**inputs):
    if "nc" not in _CACHE:
        _CACHE["nc"] = build_program()
    nc = _CACHE["nc"]
    in_maps, metas = [], []
    for c in range(8):
        m, meta = make_core_inputs(inputs, c)
        in_maps.append(m)
        metas.append(meta)
    res = run_bass_kernel_spmd(nc, in_maps, list(range(8)))
    x = np.asarray(inputs["x"], np.float32)
    out = x.copy()
    for c in range(8):
        b, d = metas[c]
        po = np.asarray(res.results[c]["pout"], np.float32)
        if d == 1:
            po = po[:, ::-1]
        out[b] += po.reshape(CH, H, W)
    return out


# revision 6
# speedup vs baseline: 1.0920x; 1.0149x over previous
"""Bidirectional spatial Mamba block on 8 Trainium2 NeuronCores — v2.

Sharding: core c = b*4 + dir*2 + half handles batch b, scan direction dir
(backward cores get host-reversed input; host un-reverses their output),
and d-half `half` of the DIN=192 inner channels. Each core runs an identical
SPMD program producing a [96, L] partial of out_w @ y_dir; the host sums the
four partials per batch and adds the residual x.

v2 design (vs baseline):
- (d,n) packed scan: the 96 d-channels x 16 states = 1536 recurrence rows are
  packed into 12 tiles of 128 partitions (p = 16*q + n, d = 8k + q), so each
  chunk runs 12 tensor_tensor_scans of [128, T] instead of 16 of [96, T].
- bf16 everywhere except the scan decay path (delta kept fp32-relative).
- All matmuls bf16 (fp32 matmul is 2 instructions and ~3x slower).
- GpSimd left idle: Pool ops contend with DVE for the SBUF port and slow
  the scans down ~40%.
- LN affine, conv bias, and the -mu*rstd LN term are folded into the fused
  in_proj+conv matmul via two extra input rows (ones row, mrs row).
- B/C/dl broadcasts into the packed layout via PE selector matmuls;
  dx replication via a DRAM round-trip DMA (write-side 16x replication).
- y = sum_n C_n*h_n accumulated in PSUM by 12 selector matmuls + a diag(D)
  matmul (the D*xc term), so no elementwise adds are needed.
"""
import numpy as np

import concourse.bass as bass
import concourse.mybir as mybir
import concourse.tile as tile
from concourse.bass_utils import run_bass_kernel_spmd

AF = mybir.ActivationFunctionType
OP = mybir.AluOpType
FP32 = mybir.dt.float32
BF16 = mybir.dt.bfloat16

CH, DIN, NST, DTR, DCONV = 96, 192, 16, 6, 4
B, H, W = 2, 128, 128
L = H * W
T = 512
NK = 12          # (d,n) tiles of 128 partitions: 96*16 / 128
NBC = DTR + 2 * NST   # 38 x_proj rows


# ---------------------------------------------------------------- tile patch
# This walrus codegen rejects the multi-wait Drain that TileContext emits at
# exit ("Too many sync wait commands"); split the waits onto single-wait NoOps.
_PATCHED = False


def _patch_tile_drain():
    global _PATCHED
    if _PATCHED:
        return
    _PATCHED = True
    from bass_rust import ScopedClock

    def patched(self, tick_clock, wait_clock):
        nc = self.nc
        carrier = nc.sync.nop()
        wait_clock.add_sem_waits(
            carrier.ins, ScopedClock({None: tick_clock.global_clock})
        )
        si = carrier.ins.sync_info
        waits = list(si.on_wait) if si is not None else []
        if si is not None:
            si.on_wait = waits[:1]
            for w in waits[1:]:
                n2 = nc.sync.nop()
                n2.ins.sync_info = mybir.SyncInfo(on_wait=[w], on_update=[])
        nc.sync.drain()
        nc.all_engine_barrier()
        assert self.sems is not None
        popped = nc._tile_sem_poison_stack.pop()
        assert popped is self._sem_poison
        nc.clear_and_free_semaphores(list(self.sems.allocated().values()))
        nc.all_engine_barrier()

    tile.TileContext._drain_and_barrier = patched


def _split_waits(nc, max_waits=1):
    """Walrus rejects instructions carrying more than ~1 sem wait. Hoist
    extras onto same-engine NoOp carriers inserted just before."""
    for bb in nc.main_func.blocks:
        new_insts = []
        for ins in bb.instructions:
            si = ins.sync_info
            if si is not None and len(si.on_wait) > max_waits:
                waits = list(si.on_wait)
                for w in waits[max_waits:]:
                    nop = mybir.InstNoOp(
                        name=nc.get_next_instruction_name(),
                        engine=ins.engine, ins=[], outs=[],
                        sync_info=mybir.SyncInfo(on_wait=[w], on_update=[]),
                    )
                    nc.register_instruction(nop)
                    new_insts.append(nop)
                si.on_wait = waits[:max_waits]
            new_insts.append(ins)
        bb.instructions[:] = new_insts


# ---------------------------------------------------------------- builder
def build_program(nchunk=L // T):
    _patch_tile_drain()
    nc = bass.Bass(num_devices=8)
    nc.allow_non_contiguous_dma("broadcast/replication DMAs")
    # const AP so activation(bias=const) lowers
    eps_t = nc.alloc_sbuf_tensor("const-f32-lneps", [128, 1], FP32)
    nc.gpsimd.memset(eps_t.ap(), 1e-5)
    nc.const_aps.aps[(FP32, 1e-5)] = eps_t.ap()
    one_t = nc.alloc_sbuf_tensor("const-f32-one", [128, 1], FP32)
    nc.gpsimd.memset(one_t.ap(), 1.0)
    nc.const_aps.aps[(FP32, 1.0)] = one_t.ap()
    nc.all_engine_barrier()
    Ltot = nchunk * T

    din = {}
    for name, shape, dt in [
        ("xin", [CH, Ltot], BF16),
        ("wcv", [98, 8 * 96], BF16),      # fused in_proj+conv lhsT, 4 taps x 2 halves
        ("wz", [98, 96], BF16),
        ("wxp", [96, 2 * NBC], BF16),     # x_proj lhsT, K-split halves
        ("wdt", [DTR, 96], BF16),
        ("dtb", [CH, 1], FP32),
        ("seldl", [96, NK * 128], BF16),  # d -> packed(p) replication
        ("selbc", [NBC, 2 * 128], BF16),  # B,C row -> packed(p)
        ("ascv", [128, NK], FP32),        # A[d(p,k), n(p)]
        ("dvm", [96, 96], BF16),          # diag(D)
        ("sely", [128, NK * 96], BF16),   # packed(p) -> d contraction
        ("wout", [96, 96], BF16),
        ("ones1", [CH, 1], BF16),
        ("ones_r", [1, CH], BF16),
    ]:
        din[name] = nc.declare_dram_parameter(name, shape, dt, isOutput=False)
    pout = nc.declare_dram_parameter("pout", [CH, Ltot], BF16, isOutput=True)
    # dram scratch for dx replication round-trip (write-side 16x replication)
    dxscr = nc.declare_dram_parameter("dxscr", [NK * 128, T], BF16, isOutput=True)

    with tile.TileContext(nc) as tc:
        with (
            tc.tile_pool(name="const", bufs=1) as const,
            tc.tile_pool(name="io", bufs=3) as io,
            tc.tile_pool(name="work", bufs=2) as work,
            tc.tile_pool(name="small", bufs=2) as small,
            tc.tile_pool(name="big", bufs=2) as big,
            tc.tile_pool(name="ps_a", bufs=2, space="PSUM") as ps_a,
            tc.tile_pool(name="ps_mm", bufs=2, space="PSUM") as ps_mm,
            tc.tile_pool(name="ps_proj", bufs=2, space="PSUM") as ps_proj,
            tc.tile_pool(name="ps_po", bufs=1, space="PSUM") as ps_po,
            tc.tile_pool(name="ps_y", bufs=1, space="PSUM") as ps_y,
        ):
            cst = {}
            for name in ["wcv", "wz", "wxp", "wdt", "dtb", "seldl", "selbc",
                         "ascv", "dvm", "sely", "wout", "ones1", "ones_r"]:
                t = const.tile(list(din[name].shape), din[name].dtype,
                               tag=name, name=name)
                nc.gpsimd.dma_start(t[:], din[name].ap()[:])
                cst[name] = t

            # persistent ping-pong xn tiles [98, T+3]: rows 0:96 = xhat,
            # row 96 = mu*sum(gam*W) fold (mrs), row 97 = ones (bias fold)
            xn_tiles = [const.tile([98, T + 3], BF16, tag=f"xn{i}",
                                   name=f"xn{i}") for i in range(2)]
            for t_ in xn_tiles:
                nc.vector.memset(t_[:], 0.0)
                nc.vector.memset(t_[96:98, 3:T + 3], 1.0)
            # hn ping-pong [128, NK*T]
            hn_tiles = [const.tile([128, NK * T], BF16, tag=f"hn{i}",
                                   name=f"hn{i}") for i in range(2)]
            nc.vector.memset(hn_tiles[1][:], 0.0)

            # cross-stage state; emission order IS dependency order in Tile,
            # so every consumer is emitted after its producer. Front ops of
            # chunk ci are woven between the scans of chunk ci-2 such that
            # each engine stream rarely waits.
            stash = {}

            def seg_a(ci):
                """LN head: input DMA + sums (Act/PE)."""
                sl = bass.ts(ci, T)
                xt = io.tile([CH, T], BF16, tag="xt")
                nc.gpsimd.dma_start(xt[:], din["xin"].ap()[:, sl])
                xsq = work.tile([CH, T], BF16, tag="xsq")
                nc.scalar.square(xsq[:], xt[:])
                s1 = ps_a.tile([1, T], FP32, tag="psa")
                nc.tensor.matmul(s1[:], cst["ones1"][:], xt[:])
                musq = small.tile([1, T], FP32, tag="musq")
                nc.scalar.activation(musq[:], s1[:], AF.Square, scale=1.0 / CH)
                s2 = ps_a.tile([1, T], FP32, tag="psa")
                nc.tensor.matmul(s2[:], cst["ones1"][:], xsq[:])
                stash[("f", ci)] = f = {}
                f.update(xt=xt, s1=s1, s2=s2, musq=musq)

            def seg_var(ci):
                f = stash[("f", ci)]
                var = small.tile([1, T], FP32, tag="var")
                nc.vector.scalar_tensor_tensor(
                    var[:], f["s2"][:], 1.0 / CH, f["musq"][:],
                    OP.mult, OP.subtract)
                f["var"] = var

            def seg_rstd(ci):
                f = stash[("f", ci)]
                lv = small.tile([1, T], FP32, tag="lv")
                nc.scalar.activation(lv[:], f["var"][:], AF.Ln, bias=1e-5)
                rstd = small.tile([1, T], BF16, tag="rstd")
                nc.scalar.activation(rstd[:], lv[:], AF.Exp, scale=-0.5)
                rstd_b = ps_a.tile([96, T], FP32, tag="psa")
                nc.tensor.matmul(rstd_b[:], cst["ones_r"][:], rstd[:])
                rstd_bc = work.tile([96, T], BF16, tag="rstd_bc")
                nc.scalar.copy(rstd_bc[:], rstd_b[:])
                f.update(rstd=rstd, rstd_bc=rstd_bc)

            def seg_xn(ci):
                """mrs row, xhat rows, halo (DVE)."""
                f = stash[("f", ci)]
                xn = xn_tiles[ci % 2]
                xn_nxt = xn_tiles[(ci + 1) % 2]
                nc.vector.tensor_mul(xn[96:97, 3:T + 3], f["s1"][:], f["rstd"][:])
                nc.vector.tensor_mul(xn[0:96, 3:T + 3], f["xt"][:], f["rstd_bc"][:])
                nc.vector.tensor_copy(xn_nxt[:, 0:3], xn[:, T:T + 3])

            def seg_conv(ci):
                """conv/z matmuls + the three sigmoid chains (PE/Act)."""
                f = stash[("f", ci)]
                xn = xn_tiles[ci % 2]
                cps = []
                for hf in range(2):
                    cp = ps_mm.tile([96, T], FP32, tag="mm")
                    for j in range(4):
                        nc.tensor.matmul(
                            cp[:],
                            cst["wcv"][:, (4 * hf + j) * 96:(4 * hf + j + 1) * 96],
                            xn[:, j:j + T],
                            start=(j == 0), stop=(j == 3))
                    cps.append(cp)
                zps = ps_mm.tile([96, T], FP32, tag="mm")
                nc.tensor.matmul(zps[:], cst["wz"][:], xn[:, 3:T + 3])
                sgs, xbs = [], []
                for idx, ps in enumerate(cps + [zps]):
                    en1 = work.tile([96, T], FP32, tag="en1",
                                    name=f"en1_{idx}", bufs=2)
                    nc.scalar.activation(en1[:], ps[:], AF.Exp, scale=-1.0)
                    nc.scalar.activation(en1[:], en1[:], AF.Ln, bias=1.0)
                    sg = work.tile([96, T], BF16, tag="sg",
                                   name=f"sg_{idx}", bufs=3)
                    nc.scalar.activation(sg[:], en1[:], AF.Exp, scale=-1.0)
                    xb = work.tile([96, T], BF16, tag="xb",
                                   name=f"xb_{idx}", bufs=3)
                    nc.scalar.copy(xb[:], ps[:])
                    sgs.append(sg)
                    xbs.append(xb)
                f.update(sgs=sgs, xbs=xbs)

            def seg_silu_mul(ci):
                f = stash[("f", ci)]
                xcA = work.tile([96, T], BF16, tag="xcA", bufs=3)
                xcB = work.tile([96, T], BF16, tag="xcB", bufs=2)
                zs = work.tile([96, T], BF16, tag="zs", bufs=3)
                nc.vector.tensor_mul(xcA[:], f["xbs"][0][:], f["sgs"][0][:])
                nc.vector.tensor_mul(xcB[:], f["xbs"][1][:], f["sgs"][1][:])
                nc.vector.tensor_mul(zs[:], f["xbs"][2][:], f["sgs"][2][:])
                f.update(xcA=xcA, xcB=xcB, zs=zs)

            def seg_proj(ci):
                """x_proj + delta chain (PE/Act)."""
                f = stash[("f", ci)]
                dbl = ps_proj.tile([NBC, T], FP32, tag="proj")
                nc.tensor.matmul(dbl[:], cst["wxp"][:, 0:NBC], f["xcA"][:],
                                 start=True, stop=False)
                nc.tensor.matmul(dbl[:], cst["wxp"][:, NBC:2 * NBC], f["xcB"][:],
                                 start=False, stop=True)
                dblb = work.tile([NBC, T], BF16, tag="dblb")
                nc.scalar.copy(dblb[:], dbl[:])
                dpre = ps_proj.tile([96, T], FP32, tag="proj")
                nc.tensor.matmul(dpre[:], cst["wdt"][:], dblb[0:DTR, :])
                spe = work.tile([96, T], FP32, tag="spe")
                nc.scalar.activation(spe[:], dpre[:], AF.Exp, bias=cst["dtb"][:])
                dlb = work.tile([96, T], BF16, tag="dlb")
                nc.scalar.activation(dlb[:], spe[:], AF.Ln, bias=1.0)
                f.update(dblb=dblb, dlb=dlb)

            def seg_dx(ci):
                """dx mul (DVE), replication DMAs, B/C selector matmuls."""
                f = stash[("f", ci)]
                dxb = work.tile([96, T], BF16, tag="dxb")
                nc.vector.tensor_mul(dxb[:], f["dlb"][:], f["xcA"][:])
                wsrc2 = dxb[:].unsqueeze(1).broadcast_to([96, 16, T])
                wdst = dxscr.ap().rearrange("(d r) t -> d r t", r=16)
                nc.sync.dma_start(wdst, wsrc2)
                dx_rep = big.tile([128, NK * T], BF16, tag="dx_rep", bufs=2)
                rsrc = dxscr.ap().rearrange("(k p) t -> p k t", p=128)
                rdst = dx_rep[:].rearrange("p (k t) -> p k t", k=NK)
                nc.sync.dma_start(rdst, rsrc)
                brep_ps = ps_a.tile([128, T], FP32, tag="psa")
                nc.tensor.matmul(brep_ps[:], cst["selbc"][:, 0:128], f["dblb"][:])
                brep = work.tile([128, T], BF16, tag="brep")
                nc.scalar.copy(brep[:], brep_ps[:])
                crep_ps = ps_a.tile([128, T], FP32, tag="psa")
                nc.tensor.matmul(crep_ps[:], cst["selbc"][:, 128:256], f["dblb"][:])
                crep = work.tile([128, T], BF16, tag="crep", bufs=3)
                nc.scalar.copy(crep[:], crep_ps[:])
                f.update(dxb=dxb, dx_rep=dx_rep, brep=brep, crep=crep)

            def seg_bn(ci):
                f = stash[("f", ci)]
                bn_all = big.tile([128, NK * T], BF16, tag="bn_all", bufs=2)
                nc.vector.tensor_mul(
                    bn_all[:].rearrange("p (k t) -> p k t", k=NK),
                    f["dx_rep"][:].rearrange("p (k t) -> p k t", k=NK),
                    f["brep"][:].unsqueeze(1).broadcast_to([128, NK, T]),
                )
                f["bn"] = bn_all

            def seg_an(ci, ks):
                f = stash[("f", ci)]
                if "an" not in f:
                    f["an"] = big.tile([128, NK * T], FP32, tag="an_all",
                                       bufs=2, name="an_all")
                an_all = f["an"]
                for k in ks:
                    dlrep = ps_a.tile([128, T], FP32, tag="psa")
                    nc.tensor.matmul(
                        dlrep[:], cst["seldl"][:, 128 * k:128 * (k + 1)],
                        f["dlb"][:])
                    nc.scalar.activation(
                        an_all[:, k * T:(k + 1) * T], dlrep[:], AF.Exp,
                        scale=cst["ascv"][:, k:k + 1])

            def scans(ci, ks):
                f = stash[("f", ci)]
                hn_all = hn_tiles[ci % 2]
                hn_prev = hn_tiles[(ci + 1) % 2]
                for k in ks:
                    init = (0.0 if ci == 0
                            else hn_prev[:, (k + 1) * T - 1:(k + 1) * T])
                    nc.vector.tensor_tensor_scan(
                        hn_all[:, k * T:(k + 1) * T],
                        f["an"][:, k * T:(k + 1) * T],
                        f["bn"][:, k * T:(k + 1) * T], init, OP.mult, OP.add)

            def back_pn(ci, half):
                f = stash[("f", ci)]
                hn_all = hn_tiles[ci % 2]
                if half == 0:
                    f["pn"] = big.tile([128, NK * T], BF16, tag="pn_all",
                                       bufs=1, name="pn_all")
                pn, crep = f["pn"], f["crep"]
                h = NK // 2
                s = slice(half * h * T, (half + h * half + (1 - half) * h) * T)
                ks = range(half * h, half * h + h)
                nc.vector.tensor_mul(
                    pn[:, half * h * T:(half * h + h) * T].rearrange(
                        "p (k t) -> p k t", k=h),
                    hn_all[:, half * h * T:(half * h + h) * T].rearrange(
                        "p (k t) -> p k t", k=h),
                    crep[:].unsqueeze(1).broadcast_to([128, h, T]),
                )

            def back_yacc(ci, half):
                f = stash[("f", ci)]
                if half == 0:
                    yacc = ps_y.tile([96, T], FP32, tag="y")
                    nc.tensor.matmul(yacc[:], cst["dvm"][:], f["xcA"][:],
                                     start=True, stop=False)
                    f["yacc"] = yacc
                    ks = range(0, NK // 2)
                else:
                    yacc = f["yacc"]
                    ks = range(NK // 2, NK)
                for k in ks:
                    nc.tensor.matmul(
                        yacc[:], cst["sely"][:, 96 * k:96 * (k + 1)],
                        f["pn"][:, k * T:(k + 1) * T],
                        start=False, stop=(k == NK - 1))

            def back_y2(ci):
                f = stash[("f", ci)]
                y2 = work.tile([96, T], BF16, tag="y2", bufs=3)
                nc.vector.tensor_mul(y2[:], f["yacc"][:], f["zs"][:])
                f["y2"] = y2

            def back_fin(ci):
                f = stash.pop(("f", ci))
                sl = bass.ts(ci, T)
                po = ps_po.tile([96, T], FP32, tag="po")
                nc.tensor.matmul(po[:], cst["wout"][:], f["y2"][:])
                pos = work.tile([96, T], BF16, tag="pos")
                nc.scalar.copy(pos[:], po[:])
                nc.gpsimd.dma_start(pout.ap()[:, sl], pos[:])

            seg_a(0)
            for ci in range(nchunk + 3):
                f1 = ci < nchunk          # front part 1 of chunk ci
                f2 = 0 <= ci - 1 < nchunk  # front part 2 of chunk ci-1
                bk = 2 <= ci < nchunk + 2  # back phase of chunk ci-2
                if ci >= 3:
                    back_fin(ci - 3)       # output tail of chunk ci-3
                if f2:
                    seg_proj(ci - 1)
                if bk:
                    scans(ci - 2, range(0, 4))
                if f1:
                    seg_var(ci)
                    seg_rstd(ci)
                if bk:
                    scans(ci - 2, range(4, 6))
                    back_pn(ci - 2, 0)
                if f2:
                    seg_dx(ci - 1)
                if bk:
                    back_yacc(ci - 2, 0)
                if f1:
                    seg_xn(ci)
                    seg_conv(ci)
                if bk:
                    scans(ci - 2, range(6, 10))
                    scans(ci - 2, range(10, NK))
                    back_pn(ci - 2, 1)
                    back_yacc(ci - 2, 1)
                if f2:
                    seg_bn(ci - 1)
                if f1:
                    seg_silu_mul(ci)
                if f2:
                    seg_an(ci - 1, range(0, 3))
                if ci + 1 < nchunk:
                    seg_a(ci + 1)
                if f2:
                    seg_an(ci - 1, range(3, NK))
                if bk:
                    back_y2(ci - 2)
    _split_waits(nc)
    return nc


# ---------------------------------------------------------------- host side
def _bf(x):
    import ml_dtypes
    return np.asarray(x, dtype=ml_dtypes.bfloat16)


def make_core_inputs(inputs, c, ltot=L):
    b, d, half = c // 4, (c // 2) % 2, c % 2
    hs = slice(half * 96, half * 96 + 96)
    oth = slice((1 - half) * 96, (1 - half) * 96 + 96)
    x = np.asarray(inputs["x"], np.float32)
    xb = x[b].reshape(CH, L)[:, :ltot]
    if d == 1:
        xb = xb[:, ::-1]
    pfx = "f_" if d == 0 else "b_"
    g = lambda n: np.asarray(inputs[pfx + n], np.float32)
    in_w = g("in_w")
    conv_w = g("conv_w")[:, 0, :]
    conv_b = g("conv_b")
    xproj_w = g("xproj_w")
    dt_w = g("dt_w")
    dt_b = g("dt_b")
    A = -np.exp(g("A_log"))
    D = g("D")
    out_w = np.asarray(inputs["out_w"], np.float32)
    gam = np.asarray(inputs["ln_g"], np.float32)
    bet = np.asarray(inputs["ln_b"], np.float32)

    # fused conv+in_proj lhsT [98, 8*96]: tap j of half X
    wcv = np.zeros((98, 8 * 96), np.float32)
    for hf, sel in ((0, hs), (1, oth)):
        Win = in_w[sel]                     # [96 d, 96 c]
        cw = conv_w[sel]                    # [96 d, 4]
        cb = conv_b[sel]
        Wb = Win @ bet                      # [96]
        Wg1 = Win @ gam * 0  # placeholder
        rowsum = (Win * gam[None, :]).sum(1)   # sum_c gamma_c Win[d,c]
        for j in range(4):
            col = (4 * hf + j) * 96
            wcv[0:96, col:col + 96] = (gam[:, None] * Win.T) * cw[:, j][None, :]
            wcv[97, col:col + 96] = cw[:, j] * Wb + (cb if j == 3 else 0.0)
            wcv[96, col:col + 96] = -cw[:, j] * rowsum / CH
    # z lhsT
    Wz = in_w[192 + half * 96:192 + half * 96 + 96]
    wz = np.zeros((98, 96), np.float32)
    wz[0:96] = gam[:, None] * Wz.T
    wz[97] = Wz @ bet
    wz[96] = -(Wz * gam[None, :]).sum(1) / CH

    # x_proj lhsT, K-split [96, 2*38]
    wxp = np.zeros((96, 2 * NBC), np.float32)
    wxp[:, 0:NBC] = xproj_w[:, hs].T
    wxp[:, NBC:] = xproj_w[:, oth].T

    wdt = dt_w[hs].T                        # [6, 96]
    dtb = dt_b[hs][:, None]

    # packed-layout selectors: p = 16*q + n, d = 8k + q
    seldl = np.zeros((96, NK * 128), np.float32)
    sely = np.zeros((128, NK * 96), np.float32)
    ascv = np.zeros((128, NK), np.float32)
    Ah = A[hs]                              # [96, 16]
    for k in range(NK):
        for p in range(128):
            q, n = p // 16, p % 16
            dloc = 8 * k + q
            seldl[dloc, 128 * k + p] = 1.0
            sely[p, 96 * k + dloc] = 1.0
            ascv[p, k] = Ah[dloc, n]
    selbc = np.zeros((NBC, 2 * 128), np.float32)
    for p in range(128):
        n = p % 16
        selbc[DTR + n, p] = 1.0
        selbc[DTR + NST + n, 128 + p] = 1.0

    return {
        "xin": _bf(xb),
        "wcv": _bf(wcv),
        "wz": _bf(wz),
        "wxp": _bf(wxp),
        "wdt": _bf(wdt),
        "dtb": np.ascontiguousarray(dtb, np.float32),
        "seldl": _bf(seldl),
        "selbc": _bf(selbc),
        "ascv": np.ascontiguousarray(ascv, np.float32),
        "dvm": _bf(np.diag(D[hs])),
        "sely": _bf(sely),
        "wout": _bf(out_w[:, hs].T),
        "ones1": _bf(np.ones((CH, 1))),
        "ones_r": _bf(np.ones((1, CH))),
    }, (b, d)


_CACHE = {}


def kernel(**inputs):
    if "nc" not in _CACHE:
        _CACHE["nc"] = build_program()
    nc = _CACHE["nc"]
    in_maps, metas = [], []
    for c in range(8):
        m, meta = make_core_inputs(inputs, c)
        in_maps.append(m)
        metas.append(meta)
    res = run_bass_kernel_spmd(nc, in_maps, list(range(8)))
    x = np.asarray(inputs["x"], np.float32)
    out = x.copy()
    for c in range(8):
        b, d = metas[c]
        po = np.asarray(res.results[c]["pout"], np.float32)
        if d == 1:
            po = po[:, ::-1]
        out[b] += po.reshape(CH, H, W)
    return out


# revision 7
# speedup vs baseline: 1.0937x; 1.0016x over previous
"""Bidirectional spatial Mamba block on 8 Trainium2 NeuronCores — v2.

Sharding: core c = b*4 + dir*2 + half handles batch b, scan direction dir
(backward cores get host-reversed input; host un-reverses their output),
and d-half `half` of the DIN=192 inner channels. Each core runs an identical
SPMD program producing a [96, L] partial of out_w @ y_dir; the host sums the
four partials per batch and adds the residual x.

v2 design (vs baseline):
- (d,n) packed scan: the 96 d-channels x 16 states = 1536 recurrence rows are
  packed into 12 tiles of 128 partitions (p = 16*q + n, d = 8k + q), so each
  chunk runs 12 tensor_tensor_scans of [128, T] instead of 16 of [96, T].
- bf16 everywhere except the scan decay path (delta kept fp32-relative).
- All matmuls bf16 (fp32 matmul is 2 instructions and ~3x slower).
- GpSimd left idle: Pool ops contend with DVE for the SBUF port and slow
  the scans down ~40%.
- LN affine, conv bias, and the -mu*rstd LN term are folded into the fused
  in_proj+conv matmul via two extra input rows (ones row, mrs row).
- B/C/dl broadcasts into the packed layout via PE selector matmuls;
  dx replication via a DRAM round-trip DMA (write-side 16x replication).
- y = sum_n C_n*h_n accumulated in PSUM by 12 selector matmuls + a diag(D)
  matmul (the D*xc term), so no elementwise adds are needed.
"""
import numpy as np

import concourse.bass as bass
import concourse.mybir as mybir
import concourse.tile as tile
from concourse.bass_utils import run_bass_kernel_spmd

AF = mybir.ActivationFunctionType
OP = mybir.AluOpType
FP32 = mybir.dt.float32
BF16 = mybir.dt.bfloat16

CH, DIN, NST, DTR, DCONV = 96, 192, 16, 6, 4
B, H, W = 2, 128, 128
L = H * W
T = 512
NK = 12          # (d,n) tiles of 128 partitions: 96*16 / 128
NBC = DTR + 2 * NST   # 38 x_proj rows


# ---------------------------------------------------------------- tile patch
# This walrus codegen rejects the multi-wait Drain that TileContext emits at
# exit ("Too many sync wait commands"); split the waits onto single-wait NoOps.
_PATCHED = False


def _patch_tile_drain():
    global _PATCHED
    if _PATCHED:
        return
    _PATCHED = True
    from bass_rust import ScopedClock

    def patched(self, tick_clock, wait_clock):
        nc = self.nc
        carrier = nc.sync.nop()
        wait_clock.add_sem_waits(
            carrier.ins, ScopedClock({None: tick_clock.global_clock})
        )
        si = carrier.ins.sync_info
        waits = list(si.on_wait) if si is not None else []
        if si is not None:
            si.on_wait = waits[:1]
            for w in waits[1:]:
                n2 = nc.sync.nop()
                n2.ins.sync_info = mybir.SyncInfo(on_wait=[w], on_update=[])
        nc.sync.drain()
        nc.all_engine_barrier()
        assert self.sems is not None
        popped = nc._tile_sem_poison_stack.pop()
        assert popped is self._sem_poison
        nc.clear_and_free_semaphores(list(self.sems.allocated().values()))
        nc.all_engine_barrier()

    tile.TileContext._drain_and_barrier = patched


def _split_waits(nc, max_waits=1):
    """Walrus rejects instructions carrying more than ~1 sem wait. Hoist
    extras onto same-engine NoOp carriers inserted just before."""
    for bb in nc.main_func.blocks:
        new_insts = []
        for ins in bb.instructions:
            si = ins.sync_info
            if si is not None and len(si.on_wait) > max_waits:
                waits = list(si.on_wait)
                for w in waits[max_waits:]:
                    nop = mybir.InstNoOp(
                        name=nc.get_next_instruction_name(),
                        engine=ins.engine, ins=[], outs=[],
                        sync_info=mybir.SyncInfo(on_wait=[w], on_update=[]),
                    )
                    nc.register_instruction(nop)
                    new_insts.append(nop)
                si.on_wait = waits[:max_waits]
            new_insts.append(ins)
        bb.instructions[:] = new_insts


# ---------------------------------------------------------------- builder
def build_program(nchunk=L // T):
    _patch_tile_drain()
    nc = bass.Bass(num_devices=8)
    nc.allow_non_contiguous_dma("broadcast/replication DMAs")
    # const AP so activation(bias=const) lowers
    eps_t = nc.alloc_sbuf_tensor("const-f32-lneps", [128, 1], FP32)
    nc.gpsimd.memset(eps_t.ap(), 1e-5)
    nc.const_aps.aps[(FP32, 1e-5)] = eps_t.ap()
    one_t = nc.alloc_sbuf_tensor("const-f32-one", [128, 1], FP32)
    nc.gpsimd.memset(one_t.ap(), 1.0)
    nc.const_aps.aps[(FP32, 1.0)] = one_t.ap()
    nc.all_engine_barrier()
    Ltot = nchunk * T

    din = {}
    for name, shape, dt in [
        ("xin", [CH, Ltot], BF16),
        ("wcv", [98, 8 * 96], BF16),      # fused in_proj+conv lhsT, 4 taps x 2 halves
        ("wz", [98, 96], BF16),
        ("wxp", [96, 2 * NBC], BF16),     # x_proj lhsT, K-split halves
        ("wdt", [DTR, 96], BF16),
        ("dtb", [CH, 1], FP32),
        ("seldl", [96, NK * 128], BF16),  # d -> packed(p) replication
        ("selbc", [NBC, 2 * 128], BF16),  # B,C row -> packed(p)
        ("ascv", [128, NK], FP32),        # A[d(p,k), n(p)]
        ("dvm", [96, 96], BF16),          # diag(D)
        ("sely", [128, NK * 96], BF16),   # packed(p) -> d contraction
        ("wout", [96, 96], BF16),
        ("ones1", [CH, 1], BF16),
        ("ones_r", [1, CH], BF16),
    ]:
        din[name] = nc.declare_dram_parameter(name, shape, dt, isOutput=False)
    pout = nc.declare_dram_parameter("pout", [CH, Ltot], BF16, isOutput=True)
    # dram scratch for dx replication round-trip (write-side 16x replication)
    dxscr = nc.declare_dram_parameter("dxscr", [NK * 128, T], BF16, isOutput=True)

    with tile.TileContext(nc) as tc:
        with (
            tc.tile_pool(name="const", bufs=1) as const,
            tc.tile_pool(name="io", bufs=3) as io,
            tc.tile_pool(name="work", bufs=2) as work,
            tc.tile_pool(name="small", bufs=2) as small,
            tc.tile_pool(name="big", bufs=2) as big,
            tc.tile_pool(name="ps_a", bufs=2, space="PSUM") as ps_a,
            tc.tile_pool(name="ps_mm", bufs=1, space="PSUM") as ps_mm,
            tc.tile_pool(name="ps_proj", bufs=2, space="PSUM") as ps_proj,
            tc.tile_pool(name="ps_pair", bufs=1, space="PSUM") as ps_pair,
            tc.tile_pool(name="ps_y", bufs=1, space="PSUM") as ps_y,
        ):
            cst = {}
            for name in ["wcv", "wz", "wxp", "wdt", "dtb", "seldl", "selbc",
                         "ascv", "dvm", "sely", "wout", "ones1", "ones_r"]:
                t = const.tile(list(din[name].shape), din[name].dtype,
                               tag=name, name=name)
                nc.gpsimd.dma_start(t[:], din[name].ap()[:])
                cst[name] = t

            # persistent ping-pong xn tiles [98, T+3]: rows 0:96 = xhat,
            # row 96 = mu*sum(gam*W) fold (mrs), row 97 = ones (bias fold)
            xn_tiles = [const.tile([98, T + 3], BF16, tag=f"xn{i}",
                                   name=f"xn{i}") for i in range(2)]
            for t_ in xn_tiles:
                nc.vector.memset(t_[:], 0.0)
                nc.vector.memset(t_[96:98, 3:T + 3], 1.0)
            # hn ping-pong [128, NK*T]
            hn_tiles = [const.tile([128, NK * T], BF16, tag=f"hn{i}",
                                   name=f"hn{i}") for i in range(2)]
            nc.vector.memset(hn_tiles[1][:], 0.0)

            # cross-stage state; emission order IS dependency order in Tile,
            # so every consumer is emitted after its producer. Front ops of
            # chunk ci are woven between the scans of chunk ci-2 such that
            # each engine stream rarely waits.
            stash = {}

            def seg_a(ci):
                """LN head: input DMA + sums (Act/PE)."""
                sl = bass.ts(ci, T)
                xt = io.tile([CH, T], BF16, tag="xt")
                nc.gpsimd.dma_start(xt[:], din["xin"].ap()[:, sl])
                xsq = work.tile([CH, T], BF16, tag="xsq")
                nc.scalar.square(xsq[:], xt[:])
                s1 = ps_a.tile([1, T], FP32, tag="psa")
                nc.tensor.matmul(s1[:], cst["ones1"][:], xt[:])
                musq = small.tile([1, T], FP32, tag="musq")
                nc.scalar.activation(musq[:], s1[:], AF.Square, scale=1.0 / CH)
                s2 = ps_a.tile([1, T], FP32, tag="psa")
                nc.tensor.matmul(s2[:], cst["ones1"][:], xsq[:])
                stash[("f", ci)] = f = {}
                f.update(xt=xt, s1=s1, s2=s2, musq=musq)

            def seg_var(ci):
                f = stash[("f", ci)]
                var = small.tile([1, T], FP32, tag="var")
                nc.vector.scalar_tensor_tensor(
                    var[:], f["s2"][:], 1.0 / CH, f["musq"][:],
                    OP.mult, OP.subtract)
                f["var"] = var

            def seg_rstd(ci):
                f = stash[("f", ci)]
                lv = small.tile([1, T], FP32, tag="lv")
                nc.scalar.activation(lv[:], f["var"][:], AF.Ln, bias=1e-5)
                rstd = small.tile([1, T], BF16, tag="rstd")
                nc.scalar.activation(rstd[:], lv[:], AF.Exp, scale=-0.5)
                rstd_b = ps_a.tile([96, T], FP32, tag="psa")
                nc.tensor.matmul(rstd_b[:], cst["ones_r"][:], rstd[:])
                rstd_bc = work.tile([96, T], BF16, tag="rstd_bc")
                nc.scalar.copy(rstd_bc[:], rstd_b[:])
                f.update(rstd=rstd, rstd_bc=rstd_bc)

            def seg_xn(ci):
                """mrs row, xhat rows, halo (DVE)."""
                f = stash[("f", ci)]
                xn = xn_tiles[ci % 2]
                xn_nxt = xn_tiles[(ci + 1) % 2]
                nc.vector.tensor_mul(xn[96:97, 3:T + 3], f["s1"][:], f["rstd"][:])
                nc.vector.tensor_mul(xn[0:96, 3:T + 3], f["xt"][:], f["rstd_bc"][:])
                nc.vector.tensor_copy(xn_nxt[:, 0:3], xn[:, T:T + 3])

            def seg_conv(ci):
                """conv/z matmuls + the three sigmoid chains (PE/Act)."""
                f = stash[("f", ci)]
                xn = xn_tiles[ci % 2]
                cps = []
                for hf in range(2):
                    cp = ps_mm.tile([96, T], FP32, tag="mm")
                    for j in range(4):
                        nc.tensor.matmul(
                            cp[:],
                            cst["wcv"][:, (4 * hf + j) * 96:(4 * hf + j + 1) * 96],
                            xn[:, j:j + T],
                            start=(j == 0), stop=(j == 3))
                    cps.append(cp)
                zps = ps_mm.tile([96, T], FP32, tag="mm")
                nc.tensor.matmul(zps[:], cst["wz"][:], xn[:, 3:T + 3])
                sgs, xbs = [], []
                for idx, ps in enumerate(cps + [zps]):
                    en1 = work.tile([96, T], FP32, tag="en1",
                                    name=f"en1_{idx}", bufs=2)
                    nc.scalar.activation(en1[:], ps[:], AF.Exp, scale=-1.0)
                    nc.scalar.activation(en1[:], en1[:], AF.Ln, bias=1.0)
                    sg = work.tile([96, T], BF16, tag="sg",
                                   name=f"sg_{idx}", bufs=3)
                    nc.scalar.activation(sg[:], en1[:], AF.Exp, scale=-1.0)
                    xb = work.tile([96, T], BF16, tag="xb",
                                   name=f"xb_{idx}", bufs=3)
                    nc.scalar.copy(xb[:], ps[:])
                    sgs.append(sg)
                    xbs.append(xb)
                f.update(sgs=sgs, xbs=xbs)

            def seg_silu_mul(ci):
                f = stash[("f", ci)]
                xcA = work.tile([96, T], BF16, tag="xcA", bufs=3)
                xcB = work.tile([96, T], BF16, tag="xcB", bufs=2)
                zs = work.tile([96, T], BF16, tag="zs", bufs=3)
                nc.vector.tensor_mul(xcA[:], f["xbs"][0][:], f["sgs"][0][:])
                nc.vector.tensor_mul(xcB[:], f["xbs"][1][:], f["sgs"][1][:])
                nc.vector.tensor_mul(zs[:], f["xbs"][2][:], f["sgs"][2][:])
                f.update(xcA=xcA, xcB=xcB, zs=zs)

            def seg_proj(ci):
                """x_proj + delta chain (PE/Act)."""
                f = stash[("f", ci)]
                dbl = ps_proj.tile([NBC, T], FP32, tag="proj")
                nc.tensor.matmul(dbl[:], cst["wxp"][:, 0:NBC], f["xcA"][:],
                                 start=True, stop=False)
                nc.tensor.matmul(dbl[:], cst["wxp"][:, NBC:2 * NBC], f["xcB"][:],
                                 start=False, stop=True)
                dblb = work.tile([NBC, T], BF16, tag="dblb")
                nc.scalar.copy(dblb[:], dbl[:])
                dpre = ps_proj.tile([96, T], FP32, tag="proj")
                nc.tensor.matmul(dpre[:], cst["wdt"][:], dblb[0:DTR, :])
                spe = work.tile([96, T], FP32, tag="spe")
                nc.scalar.activation(spe[:], dpre[:], AF.Exp, bias=cst["dtb"][:])
                dlb = work.tile([96, T], BF16, tag="dlb")
                nc.scalar.activation(dlb[:], spe[:], AF.Ln, bias=1.0)
                f.update(dblb=dblb, dlb=dlb)

            def seg_dx(ci):
                """dx mul (DVE), replication DMAs, B/C selector matmuls."""
                f = stash[("f", ci)]
                dxb = work.tile([96, T], BF16, tag="dxb")
                nc.vector.tensor_mul(dxb[:], f["dlb"][:], f["xcA"][:])
                wsrc2 = dxb[:].unsqueeze(1).broadcast_to([96, 16, T])
                wdst = dxscr.ap().rearrange("(d r) t -> d r t", r=16)
                nc.sync.dma_start(wdst, wsrc2)
                dx_rep = big.tile([128, NK * T], BF16, tag="dx_rep", bufs=2)
                rsrc = dxscr.ap().rearrange("(k p) t -> p k t", p=128)
                rdst = dx_rep[:].rearrange("p (k t) -> p k t", k=NK)
                nc.sync.dma_start(rdst, rsrc)
                brep_ps = ps_a.tile([128, T], FP32, tag="psa")
                nc.tensor.matmul(brep_ps[:], cst["selbc"][:, 0:128], f["dblb"][:])
                brep = work.tile([128, T], BF16, tag="brep")
                nc.scalar.copy(brep[:], brep_ps[:])
                crep_ps = ps_a.tile([128, T], FP32, tag="psa")
                nc.tensor.matmul(crep_ps[:], cst["selbc"][:, 128:256], f["dblb"][:])
                crep = work.tile([128, T], BF16, tag="crep", bufs=3)
                nc.scalar.copy(crep[:], crep_ps[:])
                f.update(dxb=dxb, dx_rep=dx_rep, brep=brep, crep=crep)

            def seg_bn(ci):
                f = stash[("f", ci)]
                bn_all = big.tile([128, NK * T], BF16, tag="bn_all", bufs=2)
                nc.vector.tensor_mul(
                    bn_all[:].rearrange("p (k t) -> p k t", k=NK),
                    f["dx_rep"][:].rearrange("p (k t) -> p k t", k=NK),
                    f["brep"][:].unsqueeze(1).broadcast_to([128, NK, T]),
                )
                f["bn"] = bn_all

            def seg_an(ci, kps):
                f = stash[("f", ci)]
                if "an" not in f:
                    f["an"] = big.tile([128, NK * T], FP32, tag="an_all",
                                       bufs=2, name="an_all")
                an_all = f["an"]
                for k in kps:
                    pair = ps_pair.tile([128, 2 * T], FP32, tag="pair")
                    nc.tensor.matmul(
                        pair[:, 0:T], cst["seldl"][:, 128 * k:128 * (k + 1)],
                        f["dlb"][:])
                    nc.tensor.matmul(
                        pair[:, T:2 * T],
                        cst["seldl"][:, 128 * (k + 1):128 * (k + 2)],
                        f["dlb"][:])
                    nc.scalar.activation(
                        an_all[:, k * T:(k + 2) * T], pair[:], AF.Exp)

            def scans(ci, ks):
                f = stash[("f", ci)]
                hn_all = hn_tiles[ci % 2]
                hn_prev = hn_tiles[(ci + 1) % 2]
                for k in ks:
                    init = (0.0 if ci == 0
                            else hn_prev[:, (k + 1) * T - 1:(k + 1) * T])
                    nc.vector.tensor_tensor_scan(
                        hn_all[:, k * T:(k + 1) * T],
                        f["an"][:, k * T:(k + 1) * T],
                        f["bn"][:, k * T:(k + 1) * T], init, OP.mult, OP.add)

            def back_pn(ci, half):
                f = stash[("f", ci)]
                hn_all = hn_tiles[ci % 2]
                if half == 0:
                    f["pn"] = big.tile([128, NK * T], BF16, tag="pn_all",
                                       bufs=1, name="pn_all")
                pn, crep = f["pn"], f["crep"]
                h = NK // 2
                s = slice(half * h * T, (half + h * half + (1 - half) * h) * T)
                ks = range(half * h, half * h + h)
                nc.vector.tensor_mul(
                    pn[:, half * h * T:(half * h + h) * T].rearrange(
                        "p (k t) -> p k t", k=h),
                    hn_all[:, half * h * T:(half * h + h) * T].rearrange(
                        "p (k t) -> p k t", k=h),
                    crep[:].unsqueeze(1).broadcast_to([128, h, T]),
                )

            def back_yacc(ci, half):
                f = stash[("f", ci)]
                if half == 0:
                    yacc = ps_y.tile([96, T], FP32, tag="y")
                    nc.tensor.matmul(yacc[:], cst["dvm"][:], f["xcA"][:],
                                     start=True, stop=False)
                    f["yacc"] = yacc
                    ks = range(0, NK // 2)
                else:
                    yacc = f["yacc"]
                    ks = range(NK // 2, NK)
                for k in ks:
                    nc.tensor.matmul(
                        yacc[:], cst["sely"][:, 96 * k:96 * (k + 1)],
                        f["pn"][:, k * T:(k + 1) * T],
                        start=False, stop=(k == NK - 1))

            def back_y2(ci):
                f = stash[("f", ci)]
                y2 = work.tile([96, T], BF16, tag="y2", bufs=3)
                nc.vector.tensor_mul(y2[:], f["yacc"][:], f["zs"][:])
                f["y2"] = y2

            def back_fin(ci):
                f = stash.pop(("f", ci))
                sl = bass.ts(ci, T)
                po = ps_proj.tile([96, T], FP32, tag="proj")
                nc.tensor.matmul(po[:], cst["wout"][:], f["y2"][:])
                pos = work.tile([96, T], BF16, tag="pos")
                nc.scalar.copy(pos[:], po[:])
                nc.gpsimd.dma_start(pout.ap()[:, sl], pos[:])

            seg_a(0)
            for ci in range(nchunk + 3):
                f1 = ci < nchunk          # front part 1 of chunk ci
                f2 = 0 <= ci - 1 < nchunk  # front part 2 of chunk ci-1
                bk = 2 <= ci < nchunk + 2  # back phase of chunk ci-2
                if ci >= 3:
                    back_fin(ci - 3)       # output tail of chunk ci-3
                if f2:
                    seg_proj(ci - 1)
                if bk:
                    scans(ci - 2, range(0, 4))
                if f1:
                    seg_var(ci)
                    seg_rstd(ci)
                if bk:
                    scans(ci - 2, range(4, 6))
                    back_pn(ci - 2, 0)
                if f2:
                    seg_dx(ci - 1)
                if bk:
                    back_yacc(ci - 2, 0)
                if f1:
                    seg_xn(ci)
                    seg_conv(ci)
                if bk:
                    scans(ci - 2, range(6, 10))
                    scans(ci - 2, range(10, NK))
                    back_pn(ci - 2, 1)
                    back_yacc(ci - 2, 1)
                if f2:
                    seg_bn(ci - 1)
                if f1:
                    seg_silu_mul(ci)
                if f2:
                    seg_an(ci - 1, (0, 2))
                if ci + 1 < nchunk:
                    seg_a(ci + 1)
                if f2:
                    seg_an(ci - 1, (4, 6, 8, 10))
                if bk:
                    back_y2(ci - 2)
    _split_waits(nc)
    return nc


# ---------------------------------------------------------------- host side
def _bf(x):
    import ml_dtypes
    return np.asarray(x, dtype=ml_dtypes.bfloat16)


def make_core_inputs(inputs, c, ltot=L):
    b, d, half = c // 4, (c // 2) % 2, c % 2
    hs = slice(half * 96, half * 96 + 96)
    oth = slice((1 - half) * 96, (1 - half) * 96 + 96)
    x = np.asarray(inputs["x"], np.float32)
    xb = x[b].reshape(CH, L)[:, :ltot]
    if d == 1:
        xb = xb[:, ::-1]
    pfx = "f_" if d == 0 else "b_"
    g = lambda n: np.asarray(inputs[pfx + n], np.float32)
    in_w = g("in_w")
    conv_w = g("conv_w")[:, 0, :]
    conv_b = g("conv_b")
    xproj_w = g("xproj_w")
    dt_w = g("dt_w")
    dt_b = g("dt_b")
    A = -np.exp(g("A_log"))
    D = g("D")
    out_w = np.asarray(inputs["out_w"], np.float32)
    gam = np.asarray(inputs["ln_g"], np.float32)
    bet = np.asarray(inputs["ln_b"], np.float32)

    # fused conv+in_proj lhsT [98, 8*96]: tap j of half X
    wcv = np.zeros((98, 8 * 96), np.float32)
    for hf, sel in ((0, hs), (1, oth)):
        Win = in_w[sel]                     # [96 d, 96 c]
        cw = conv_w[sel]                    # [96 d, 4]
        cb = conv_b[sel]
        Wb = Win @ bet                      # [96]
        Wg1 = Win @ gam * 0  # placeholder
        rowsum = (Win * gam[None, :]).sum(1)   # sum_c gamma_c Win[d,c]
        for j in range(4):
            col = (4 * hf + j) * 96
            wcv[0:96, col:col + 96] = (gam[:, None] * Win.T) * cw[:, j][None, :]
            wcv[97, col:col + 96] = cw[:, j] * Wb + (cb if j == 3 else 0.0)
            wcv[96, col:col + 96] = -cw[:, j] * rowsum / CH
    # z lhsT
    Wz = in_w[192 + half * 96:192 + half * 96 + 96]
    wz = np.zeros((98, 96), np.float32)
    wz[0:96] = gam[:, None] * Wz.T
    wz[97] = Wz @ bet
    wz[96] = -(Wz * gam[None, :]).sum(1) / CH

    # x_proj lhsT, K-split [96, 2*38]
    wxp = np.zeros((96, 2 * NBC), np.float32)
    wxp[:, 0:NBC] = xproj_w[:, hs].T
    wxp[:, NBC:] = xproj_w[:, oth].T

    wdt = dt_w[hs].T                        # [6, 96]
    dtb = dt_b[hs][:, None]

    # packed-layout selectors: p = 16*q + n, d = 8k + q
    seldl = np.zeros((96, NK * 128), np.float32)
    sely = np.zeros((128, NK * 96), np.float32)
    ascv = np.zeros((128, NK), np.float32)
    Ah = A[hs]                              # [96, 16]
    for k in range(NK):
        for p in range(128):
            q, n = p // 16, p % 16
            dloc = 8 * k + q
            seldl[dloc, 128 * k + p] = Ah[dloc, n]
            sely[p, 96 * k + dloc] = 1.0
            ascv[p, k] = Ah[dloc, n]
    selbc = np.zeros((NBC, 2 * 128), np.float32)
    for p in range(128):
        n = p % 16
        selbc[DTR + n, p] = 1.0
        selbc[DTR + NST + n, 128 + p] = 1.0

    return {
        "xin": _bf(xb),
        "wcv": _bf(wcv),
        "wz": _bf(wz),
        "wxp": _bf(wxp),
        "wdt": _bf(wdt),
        "dtb": np.ascontiguousarray(dtb, np.float32),
        "seldl": _bf(seldl),
        "selbc": _bf(selbc),
        "ascv": np.ascontiguousarray(ascv, np.float32),
        "dvm": _bf(np.diag(D[hs])),
        "sely": _bf(sely),
        "wout": _bf(out_w[:, hs].T),
        "ones1": _bf(np.ones((CH, 1))),
        "ones_r": _bf(np.ones((1, CH))),
    }, (b, d)


_CACHE = {}


def kernel(**inputs):
    if "nc" not in _CACHE:
        _CACHE["nc"] = build_program()
    nc = _CACHE["nc"]
    in_maps, metas = [], []
    for c in range(8):
        m, meta = make_core_inputs(inputs, c)
        in_maps.append(m)
        metas.append(meta)
    res = run_bass_kernel_spmd(nc, in_maps, list(range(8)))
    x = np.asarray(inputs["x"], np.float32)
    out = x.copy()
    for c in range(8):
        b, d = metas[c]
        po = np.asarray(res.results[c]["pout"], np.float32)
        if d == 1:
            po = po[:, ::-1]
        out[b] += po.reshape(CH, H, W)
    return out
